# revision 30
# baseline (speedup 1.0000x reference)
"""GRUCell Trainium2 kernel: T=512, B=64, I=H=512, 8-way data parallel over B.

Strategy
--------
- Shard batch B=64 -> 8 rows per NeuronCore; weights replicated. No collectives.
- All recurrent state is kept in a transposed layout: a [128, 32] SBUF tile
  where element [p, c*8+b] = h[b, 128*c + p]  (c = H-chunk 0..3, b = local batch).
  This keeps H on partitions so every elementwise op streams only 32 columns.
- Per step t:
    * PE: psum_zr[2,8]  = sum_kc zru[kc].T @ h_kc  + I2.T @ xzr[t]   (gate preacts)
          psum_q[128,32] (4 col groups) = sum_kc Wh[kc,mc].T @ h_kc  (h_u matmul)
          psum_b[128,64] = ones.T @ zr_sig (broadcast gates across partitions)
    * ACT: zr_sig = sigmoid(psum_zr + gate_bias);  z_bcast copy; tanh
    * DVE: u = psum_q + h_u_b;  s = u * r_bcast;  q = s + xn[t];
           d = h - nt;  e = d * z_bcast;  h' = nt + e  -> written into outS slice
- xz/xr/xn input projections are precomputed on-device from a host-transposed
  input ([I, T*8] per core) with bf16 matmuls, fp32 accumulation.
- Output accumulates in SBUF ([128, T*32] bf16) and is DMA'd out in chunks;
  host code undoes the layout and returns ((T,B,H) float32, (B,H) float32).
"""

import numpy as np
import ml_dtypes
from contextlib import ExitStack

import concourse.bass as bass
import concourse.tile as tile
from concourse import bacc, mybir
from concourse.bass_utils import run_bass_kernel_spmd

AF = mybir.ActivationFunctionType

T, B, I, H = 512, 64, 512, 512
NCORES = 8
BS = B // NCORES          # 8 batch rows per core
KC = H // 128             # 4 partition chunks of the hidden dim
W = KC * BS               # 32 = width of one state slice
F32 = mybir.dt.float32
BF16 = mybir.dt.bfloat16
bf16 = ml_dtypes.bfloat16


def _emit(ctx: ExitStack, tc: "tile.TileContext", d: dict, TS: int, dbg: dict | None = None):
    nc = tc.nc
    TB = TS * BS           # flattened (t, b) count per core

    const = ctx.enter_context(tc.tile_pool(name="const", bufs=1))
    inS = const.tile([128, KC * TB], BF16, tag="inS")
    xnS = const.tile([128, TS * W], BF16, tag="xnS")
    outS = const.tile([128, TS * W], BF16, tag="outS")
    xzrS = const.tile([2, TB], BF16, tag="xzrS")   # row 0: xz, row 1: xr
    whS = const.tile([128, KC * H], BF16, tag="whS")
    wxS = const.tile([128, KC * H], BF16, tag="wxS")
    zruS = const.tile([128, KC * 2], BF16, tag="zruS")
    zrwS = const.tile([128, KC * 2], BF16, tag="zrwS")
    hubS = const.tile([128, KC], F32, tag="hubS")
    hwbS = const.tile([128, KC], F32, tag="hwbS")
    gbW = const.tile([1, 2], BF16, tag="gbW")
    onesS = const.tile([1, 512], BF16, tag="onesS")
    i2S = const.tile([2, 2], BF16, tag="i2S")
    selS = const.tile([2, 2 * 128], BF16, tag="selS")  # [:,0:128]=[1;0], [:,128:]=[0;1]
    h0S = const.tile([128, W], BF16, tag="h0S")
    nc.vector.memset(onesS[:, :], 1.0)
    nc.sync.dma_start(i2S[:, :], d["i2"][:, :])
    nc.sync.dma_start(selS[:, :], d["sel2"][:, :])

    for kc in range(KC):
        nc.sync.dma_start(inS[:, kc * TB:(kc + 1) * TB], d["inT"][kc * 128:(kc + 1) * 128, :])
        nc.sync.dma_start(whS[:, kc * H:(kc + 1) * H], d["whT"][kc * 128:(kc + 1) * 128, :])
        nc.sync.dma_start(wxS[:, kc * H:(kc + 1) * H], d["wxT"][kc * 128:(kc + 1) * 128, :])
        nc.sync.dma_start(zruS[:, kc * 2:(kc + 1) * 2], d["zruT"][kc * 128:(kc + 1) * 128, :])
        nc.sync.dma_start(zrwS[:, kc * 2:(kc + 1) * 2], d["zrwT"][kc * 128:(kc + 1) * 128, :])
    nc.sync.dma_start(hubS[:, :], d["hubT"][:, :])
    nc.sync.dma_start(hwbS[:, :], d["hwbT"][:, :])
    nc.sync.dma_start(gbW[:, :], d["gb"][:, :])
    nc.sync.dma_start(h0S[:, :], d["h0T"][:, :])

    # ---------------- input projections (xz/xr and xn), all t in parallel ----
    CT = min(512, TB)      # columns per projection tile
    NT = TB // CT
    with tc.tile_pool(name="ppq", bufs=2, space="PSUM") as ppq, \
         tc.tile_pool(name="ppz", bufs=2, space="PSUM") as ppz:
        for nt in range(NT):
            base = nt * CT
            pz = ppz.tile([2, CT], F32, tag="pz")
            for kc in range(KC):
                nc.tensor.matmul(pz[:, :], zrwS[:, 2 * kc:2 * kc + 2],
                                 inS[:, kc * TB + base: kc * TB + base + CT],
                                 start=(kc == 0), stop=False)
            # fold gate biases (w_b + u_b) in via a K=1 ones matmul
            nc.tensor.matmul(pz[:, :], gbW[:, :], onesS[0:1, 0:CT],
                             start=False, stop=True)
            nc.vector.tensor_copy(xzrS[:, base:base + CT], pz[:, :])
            for hc in range(KC):
                pq = ppq.tile([128, CT], F32, tag="pq")
                for ic in range(KC):
                    nc.tensor.matmul(pq[:, :], wxS[:, ic * H + hc * 128: ic * H + (hc + 1) * 128],
                                     inS[:, ic * TB + base: ic * TB + base + CT],
                                     start=(ic == 0), stop=(ic == KC - 1))
                # scatter into xnS[p, t*W + hc*BS + b] with per-partition bias h_w_b
                nrows = CT // BS
                t0 = base // BS
                xn_view = xnS[:, :].rearrange("p (t c b) -> p t c b", c=KC, b=BS)
                nc.vector.tensor_scalar_add(
                    xn_view[:, t0:t0 + nrows, hc, :],
                    pq[:, :].rearrange("p (t b) -> p t b", b=BS),
                    hwbS[:, hc:hc + 1])

    if dbg is not None:
        nc.sync.dma_start(dbg["xnS"], xnS[:, :])
        nc.sync.dma_start(dbg["xzrS"], xzrS[:, :])

    # ---------------- recurrence: two staggered half-batch chains ----------
    # Chain 0 owns local batch rows 0..3, chain 1 owns 4..7. Independent
    # recurrences; the scheduler overlaps chain 0's elementwise tail with
    # chain 1's matmuls. Blend sub/add run on GpSimd to offload the DVE.
    HB = BS // 2           # 4
    HW = KC * HB           # 16 = per-chain state width
    with tc.tile_pool(name="st", bufs=3) as st, \
         tc.tile_pool(name="pqp", bufs=2, space="PSUM") as pqp, \
         tc.tile_pool(name="pzp", bufs=1, space="PSUM") as pzp, \
         tc.tile_pool(name="pbp", bufs=1, space="PSUM") as pbp:

        psum_b = {}  # chain -> live broadcast psum tile

        def prev_full(t):
            return h0S[:, :] if t == 0 else outS[:, (t - 1) * W: t * W]

        def emit_mm(ch: int, t: int):
            """Gate + main matmuls, sigmoid, gate broadcasts for (ch, t)."""
            off = ch * HB
            sfx = str(ch)
            pf = prev_full(t)

            def prev_kc(kc):
                return pf[:, kc * BS + off: kc * BS + off + HB]

            # gate preactivations: [2, HB] psum, z on partition 0, r on 1
            p_zr = pzp.tile([2, HB], F32, tag="p_zr" + sfx)
            for kc in range(KC):
                nc.tensor.matmul(p_zr[:, :], zruS[:, 2 * kc:2 * kc + 2],
                                 prev_kc(kc), start=(kc == 0), stop=False)
            nc.tensor.matmul(p_zr[:, :], i2S[:, :],
                             xzrS[:, t * BS + off: t * BS + off + HB],
                             start=False, stop=True)

            zr_s = st.tile([2, HB], BF16, tag="zr" + sfx)
            nc.scalar.activation(zr_s[:, :], p_zr[:, :], AF.Sigmoid)

            # main h_u matmul: [128(h), 4(c)*HB(b)]
            p_q = pqp.tile([128, HW], F32, tag="p_q" + sfx)
            for mc in range(KC):
                for kc in range(KC):
                    nc.tensor.matmul(p_q[:, mc * HB:(mc + 1) * HB],
                                     whS[:, kc * H + mc * 128: kc * H + (mc + 1) * 128],
                                     prev_kc(kc), start=(kc == 0), stop=(kc == KC - 1))

            # broadcast z and r across partitions via PE selector matmuls.
            # High priority: must slot right after sigmoid, before the other
            # chain's main matmuls, or the tail chain stalls.
            p_b = pbp.tile([128, 2 * HW], F32, tag="p_b" + sfx)
            rhs_zr = zr_s[:, :].unsqueeze(1).broadcast_to((2, KC, HB))
            pb_v = p_b[:, :].rearrange("p (g c b) -> p g c b", g=2, b=HB)
            with tc.high_priority():
                nc.tensor.matmul(pb_v[:, 0, :, :], selS[:, 0:128], rhs_zr, start=True, stop=True)
                nc.tensor.matmul(pb_v[:, 1, :, :], selS[:, 128:256], rhs_zr, start=True, stop=True)
            psum_b[ch] = (p_q, p_b)

        def emit_tail(ch: int, t: int):
            """Elementwise tail for (ch, t): u,s,q,tanh,blend -> outS."""
            off = ch * HB
            sfx = str(ch)
            p_q, p_b = psum_b[ch]
            pf = prev_full(t)
            prev_v = pf.rearrange("p (c b) -> p c b", b=BS)[:, :, off:off + HB]

            u = st.tile([128, HW], F32, tag="u" + sfx)
            nc.vector.tensor_add(
                u[:, :].rearrange("p (c b) -> p c b", b=HB),
                p_q[:, :].rearrange("p (c b) -> p c b", b=HB),
                hubS[:, :].unsqueeze(2).broadcast_to((128, KC, HB)))
            s = st.tile([128, HW], F32, tag="s" + sfx)
            nc.vector.tensor_mul(s[:, :], u[:, :], p_b[:, HW:2 * HW])
            q = st.tile([128, HW], F32, tag="q" + sfx)
            nc.vector.tensor_add(
                q[:, :].rearrange("p (c b) -> p c b", b=HB), s[:, :].rearrange("p (c b) -> p c b", b=HB),
                xnS[:, t * W: (t + 1) * W].rearrange("p (c b) -> p c b", b=BS)[:, :, off:off + HB])
            nt_ = st.tile([128, HW], BF16, tag="nt" + sfx)
            nc.scalar.activation(nt_[:, :], q[:, :], AF.Tanh)
            d_ = st.tile([128, HW], BF16, tag="d" + sfx)
            nc.vector.tensor_sub(d_[:, :].rearrange("p (c b) -> p c b", b=HB),
                                 prev_v, nt_[:, :].rearrange("p (c b) -> p c b", b=HB))
            e_ = st.tile([128, HW], BF16, tag="e" + sfx)
            nc.vector.tensor_mul(e_[:, :], d_[:, :], p_b[:, 0:HW])
            out_v = outS[:, t * W:(t + 1) * W].rearrange("p (c b) -> p c b", b=BS)[:, :, off:off + HB]
            nc.vector.tensor_add(out_v, nt_[:, :].rearrange("p (c b) -> p c b", b=HB),
                                 e_[:, :].rearrange("p (c b) -> p c b", b=HB))

        # Antiphase schedule: while chain A's matmuls run on PE, chain B's
        # tail runs on DVE/ACT, and vice versa.
        oc = min(64, TS)
        for t in range(TS):
            emit_mm(0, t)
            if t > 0:
                emit_tail(1, t - 1)
                # both chains complete through t-1 here; flush finished window
                if t % oc == 0:
                    nc.sync.dma_start(d["outT"][:, (t - oc) * W:t * W],
                                      outS[:, (t - oc) * W:t * W])
            emit_mm(1, t)
            emit_tail(0, t)
        emit_tail(1, TS - 1)
        nc.sync.dma_start(d["outT"][:, (TS - oc) * W:TS * W],
                          outS[:, (TS - oc) * W:TS * W])


def build_nc(TS: int = T, debug_dump: bool = False):
    nc = bacc.Bacc("TRN2", target_bir_lowering=False, debug=False)
    TB = TS * BS
    d = {}
    d["inT"] = nc.dram_tensor("inT", [I, TB], BF16, kind="ExternalInput").ap()
    d["h0T"] = nc.dram_tensor("h0T", [128, W], BF16, kind="ExternalInput").ap()
    d["whT"] = nc.dram_tensor("whT", [H, H], BF16, kind="ExternalInput").ap()
    d["wxT"] = nc.dram_tensor("wxT", [I, H], BF16, kind="ExternalInput").ap()
    d["zruT"] = nc.dram_tensor("zruT", [H, 2], BF16, kind="ExternalInput").ap()
    d["zrwT"] = nc.dram_tensor("zrwT", [I, 2], BF16, kind="ExternalInput").ap()
    d["hubT"] = nc.dram_tensor("hubT", [128, KC], F32, kind="ExternalInput").ap()
    d["hwbT"] = nc.dram_tensor("hwbT", [128, KC], F32, kind="ExternalInput").ap()
    d["gb"] = nc.dram_tensor("gb", [1, 2], BF16, kind="ExternalInput").ap()
    d["i2"] = nc.dram_tensor("i2", [2, 2], BF16, kind="ExternalInput").ap()
    d["sel2"] = nc.dram_tensor("sel2", [2, 256], BF16, kind="ExternalInput").ap()
    d["outT"] = nc.dram_tensor("outT", [128, TS * W], BF16, kind="ExternalOutput").ap()

    dbg = None
    if debug_dump:
        dbg = {
            "xnS": nc.dram_tensor("dbg_xnS", [128, TS * W], BF16, kind="ExternalOutput").ap(),
            "xzrS": nc.dram_tensor("dbg_xzrS", [1, TS * 2 * BS], BF16, kind="ExternalOutput").ap(),
            "p_q": nc.dram_tensor("dbg_p_q", [128, W], F32, kind="ExternalOutput").ap(),
            "p_b": nc.dram_tensor("dbg_p_b", [128, 2 * W], F32, kind="ExternalOutput").ap(),
            "zr_s": nc.dram_tensor("dbg_zr_s", [1, 2 * BS], BF16, kind="ExternalOutput").ap(),
            "q": nc.dram_tensor("dbg_q", [128, W], F32, kind="ExternalOutput").ap(),
            "nt": nc.dram_tensor("dbg_nt", [128, W], BF16, kind="ExternalOutput").ap(),
        }

    with tile.TileContext(nc) as tc, ExitStack() as ctx:
        _emit(ctx, tc, d, TS, dbg)
    nc.compile()
    return nc


def pack_inputs(inputs: dict, TS: int = T) -> list[dict]:
    """Host-side shard + relayout. Returns per-core in_maps."""
    f32 = np.float32
    inp = np.asarray(inputs["input"], f32)
    hid = np.asarray(inputs["hidden"], f32)
    wh = np.ascontiguousarray(np.asarray(inputs["h_u_w"], f32).T).astype(bf16)
    wx = np.ascontiguousarray(np.asarray(inputs["h_w_w"], f32).T).astype(bf16)
    zru = np.stack([np.asarray(inputs["zt_u_w"], f32)[0],
                    np.asarray(inputs["rt_u_w"], f32)[0]], axis=1).astype(bf16)
    zrw = np.stack([np.asarray(inputs["zt_w_w"], f32)[0],
                    np.asarray(inputs["rt_w_w"], f32)[0]], axis=1).astype(bf16)
    hub = np.ascontiguousarray(np.asarray(inputs["h_u_b"], f32).reshape(KC, 128).T)
    hwb = np.ascontiguousarray(np.asarray(inputs["h_w_b"], f32).reshape(KC, 128).T)
    gb = np.array([[float(inputs["zt_w_b"][0]) + float(inputs["zt_u_b"][0]),
                    float(inputs["rt_w_b"][0]) + float(inputs["rt_u_b"][0])]]).astype(bf16)
    i2 = np.eye(2, dtype=bf16)
    sel2 = np.zeros((2, 256), bf16)
    sel2[0, 0:128] = 1
    sel2[1, 128:256] = 1

    in_maps = []
    for c in range(NCORES):
        sl = inp[:TS, c * BS:(c + 1) * BS, :]                     # [TS, 8, I]
        inT = np.ascontiguousarray(sl.transpose(2, 0, 1).reshape(I, TS * BS)).astype(bf16)
        h0 = hid[c * BS:(c + 1) * BS, :]                          # [8, H]
        h0T = np.ascontiguousarray(
            h0.T.reshape(KC, 128, BS).transpose(1, 0, 2).reshape(128, W)).astype(bf16)
        in_maps.append({
            "inT": inT, "h0T": h0T, "whT": wh, "wxT": wx, "zruT": zru,
            "zrwT": zrw, "hubT": hub, "hwbT": hwb, "gb": gb, "i2": i2,
            "sel2": sel2,
        })
    return in_maps


def unpack_outputs(results: list[dict], TS: int = T):
    output = np.empty((TS, B, H), np.float32)
    for c in range(NCORES):
        o = np.asarray(results[c]["outT"]).reshape(128, TS, KC, BS)
        output[:, c * BS:(c + 1) * BS, :] = \
            o.transpose(1, 3, 2, 0).reshape(TS, BS, H).astype(np.float32)
    hidden_final = output[-1].copy()
    return output, hidden_final


_NC_CACHE = {}


def kernel(**inputs):
    if T not in _NC_CACHE:
        _NC_CACHE[T] = build_nc(T)
    nc = _NC_CACHE[T]
    in_maps = pack_inputs(inputs, T)
    res = run_bass_kernel_spmd(nc, in_maps, list(range(NCORES)))
    return unpack_outputs(res.results, T)


# revision 31
# speedup vs baseline: 1.1875x; 1.1875x over previous
"""GRUCell Trainium2 kernel: T=512, B=64, I=H=512, 8-way data parallel over B.

Strategy
--------
- Shard batch B=64 -> 8 rows per NeuronCore; weights replicated. No collectives.
- All recurrent state is kept in a transposed layout: a [128, 32] SBUF tile
  where element [p, c*8+b] = h[b, 128*c + p]  (c = H-chunk 0..3, b = local batch).
  This keeps H on partitions so every elementwise op streams only 32 columns.
- Per step t:
    * PE: psum_zr[2,8]  = sum_kc zru[kc].T @ h_kc  + I2.T @ xzr[t]   (gate preacts)
          psum_q[128,32] (4 col groups) = sum_kc Wh[kc,mc].T @ h_kc  (h_u matmul)
          psum_b[128,64] = ones.T @ zr_sig (broadcast gates across partitions)
    * ACT: zr_sig = sigmoid(psum_zr + gate_bias);  z_bcast copy; tanh
    * DVE: u = psum_q + h_u_b;  s = u * r_bcast;  q = s + xn[t];
           d = h - nt;  e = d * z_bcast;  h' = nt + e  -> written into outS slice
- xz/xr/xn input projections are precomputed on-device from a host-transposed
  input ([I, T*8] per core) with bf16 matmuls, fp32 accumulation.
- Output accumulates in SBUF ([128, T*32] bf16) and is DMA'd out in chunks;
  host code undoes the layout and returns ((T,B,H) float32, (B,H) float32).
"""

import numpy as np
import ml_dtypes
from contextlib import ExitStack

import concourse.bass as bass
import concourse.tile as tile
from concourse import bacc, mybir
from concourse.bass_utils import run_bass_kernel_spmd

AF = mybir.ActivationFunctionType

T, B, I, H = 512, 64, 512, 512
NCORES = 8
BS = B // NCORES          # 8 batch rows per core
KC = H // 128             # 4 partition chunks of the hidden dim
W = KC * BS               # 32 = width of one state slice
F32 = mybir.dt.float32
BF16 = mybir.dt.bfloat16
bf16 = ml_dtypes.bfloat16


def _emit(ctx: ExitStack, tc: "tile.TileContext", d: dict, TS: int, dbg: dict | None = None):
    nc = tc.nc
    TB = TS * BS           # flattened (t, b) count per core

    const = ctx.enter_context(tc.tile_pool(name="const", bufs=1))
    inS = const.tile([128, KC * TB], BF16, tag="inS")
    xnS = const.tile([128, TS * W], BF16, tag="xnS")
    outS = const.tile([128, TS * W], BF16, tag="outS")
    xzrS = const.tile([2, TB], BF16, tag="xzrS")   # row 0: xz, row 1: xr
    whS = const.tile([128, KC * H], BF16, tag="whS")
    wxS = const.tile([128, KC * H], BF16, tag="wxS")
    zruS = const.tile([128, KC * 2], BF16, tag="zruS")
    zrwS = const.tile([128, KC * 2], BF16, tag="zrwS")
    hubS = const.tile([128, KC], F32, tag="hubS")
    hwbS = const.tile([128, KC], F32, tag="hwbS")
    gbW = const.tile([1, 2], BF16, tag="gbW")
    onesS = const.tile([1, 512], BF16, tag="onesS")
    i2S = const.tile([2, 2], BF16, tag="i2S")
    selS = const.tile([2, 2 * 128], BF16, tag="selS")  # [:,0:128]=[1;0], [:,128:]=[0;1]
    h0S = const.tile([128, W], BF16, tag="h0S")
    nc.vector.memset(onesS[:, :], 1.0)
    nc.sync.dma_start(i2S[:, :], d["i2"][:, :])
    nc.sync.dma_start(selS[:, :], d["sel2"][:, :])

    for kc in range(KC):
        nc.sync.dma_start(inS[:, kc * TB:(kc + 1) * TB], d["inT"][kc * 128:(kc + 1) * 128, :])
        nc.sync.dma_start(whS[:, kc * H:(kc + 1) * H], d["whT"][kc * 128:(kc + 1) * 128, :])
        nc.sync.dma_start(wxS[:, kc * H:(kc + 1) * H], d["wxT"][kc * 128:(kc + 1) * 128, :])
        nc.sync.dma_start(zruS[:, kc * 2:(kc + 1) * 2], d["zruT"][kc * 128:(kc + 1) * 128, :])
        nc.sync.dma_start(zrwS[:, kc * 2:(kc + 1) * 2], d["zrwT"][kc * 128:(kc + 1) * 128, :])
    nc.sync.dma_start(hubS[:, :], d["hubT"][:, :])
    nc.sync.dma_start(hwbS[:, :], d["hwbT"][:, :])
    nc.sync.dma_start(gbW[:, :], d["gb"][:, :])
    nc.sync.dma_start(h0S[:, :], d["h0T"][:, :])

    # ---------------- input projections (xz/xr and xn), all t in parallel ----
    CT = min(512, TB)      # columns per projection tile
    NT = TB // CT
    with tc.tile_pool(name="ppq", bufs=2, space="PSUM") as ppq, \
         tc.tile_pool(name="ppz", bufs=2, space="PSUM") as ppz:
        for nt in range(NT):
            base = nt * CT
            pz = ppz.tile([2, CT], F32, tag="pz")
            for kc in range(KC):
                nc.tensor.matmul(pz[:, :], zrwS[:, 2 * kc:2 * kc + 2],
                                 inS[:, kc * TB + base: kc * TB + base + CT],
                                 start=(kc == 0), stop=False)
            # fold gate biases (w_b + u_b) in via a K=1 ones matmul
            nc.tensor.matmul(pz[:, :], gbW[:, :], onesS[0:1, 0:CT],
                             start=False, stop=True)
            nc.vector.tensor_copy(xzrS[:, base:base + CT], pz[:, :])
            for hc in range(KC):
                pq = ppq.tile([128, CT], F32, tag="pq")
                for ic in range(KC):
                    nc.tensor.matmul(pq[:, :], wxS[:, ic * H + hc * 128: ic * H + (hc + 1) * 128],
                                     inS[:, ic * TB + base: ic * TB + base + CT],
                                     start=(ic == 0), stop=(ic == KC - 1))
                # scatter into xnS[p, t*W + hc*BS + b] with per-partition bias h_w_b
                nrows = CT // BS
                t0 = base // BS
                xn_view = xnS[:, :].rearrange("p (t c b) -> p t c b", c=KC, b=BS)
                nc.vector.tensor_scalar_add(
                    xn_view[:, t0:t0 + nrows, hc, :],
                    pq[:, :].rearrange("p (t b) -> p t b", b=BS),
                    hwbS[:, hc:hc + 1])

    if dbg is not None:
        nc.sync.dma_start(dbg["xnS"], xnS[:, :])
        nc.sync.dma_start(dbg["xzrS"], xzrS[:, :])

    # ---------------- recurrence (single chain, latency-optimized) ---------
    with tc.tile_pool(name="st", bufs=3) as st, \
         tc.tile_pool(name="pqp", bufs=2, space="PSUM") as pqp, \
         tc.tile_pool(name="pzp", bufs=2, space="PSUM") as pzp, \
         tc.tile_pool(name="pbp", bufs=2, space="PSUM") as pbp:
        for t in range(TS):
            prev = h0S[:, :] if t == 0 else outS[:, (t - 1) * W: t * W]

            # gate preactivations: [2, BS] psum, z on partition 0, r on 1
            p_zr = pzp.tile([2, BS], F32, tag="p_zr")
            for kc in range(KC):
                nc.tensor.matmul(p_zr[:, :], zruS[:, 2 * kc:2 * kc + 2],
                                 prev[:, kc * BS:(kc + 1) * BS],
                                 start=(kc == 0), stop=False)
            nc.tensor.matmul(p_zr[:, :], i2S[:, :], xzrS[:, t * BS:(t + 1) * BS],
                             start=False, stop=True)

            zr_s = st.tile([2, BS], BF16, tag="zr")
            nc.scalar.activation(zr_s[:, :], p_zr[:, :], AF.Sigmoid)

            # main h_u matmul, output transposed: [128(h), 4(c)*8(b)]
            p_q = pqp.tile([128, W], F32, tag="p_q")
            for mc in range(KC):
                for kc in range(KC):
                    nc.tensor.matmul(p_q[:, mc * BS:(mc + 1) * BS],
                                     whS[:, kc * H + mc * 128: kc * H + (mc + 1) * 128],
                                     prev[:, kc * BS:(kc + 1) * BS],
                                     start=(kc == 0), stop=(kc == KC - 1))

            # broadcast z and r across partitions via PE selector matmuls
            p_b = pbp.tile([128, 2 * W], F32, tag="p_b")
            rhs_zr = zr_s[:, :].unsqueeze(1).broadcast_to((2, KC, BS))
            pb_v = p_b[:, :].rearrange("p (g c b) -> p g c b", g=2, b=BS)
            nc.tensor.matmul(pb_v[:, 0, :, :], selS[:, 0:128], rhs_zr, start=True, stop=True)
            nc.tensor.matmul(pb_v[:, 1, :, :], selS[:, 128:256], rhs_zr, start=True, stop=True)

            u = st.tile([128, W], F32, tag="u")
            nc.vector.tensor_add(
                u[:, :].rearrange("p (c b) -> p c b", b=BS),
                p_q[:, :].rearrange("p (c b) -> p c b", b=BS),
                hubS[:, :].unsqueeze(2).broadcast_to((128, KC, BS)))
            s = st.tile([128, W], F32, tag="s")
            nc.vector.tensor_mul(s[:, :], u[:, :], p_b[:, W:2 * W])
            q = st.tile([128, W], F32, tag="q")
            nc.vector.tensor_add(q[:, :], s[:, :], xnS[:, t * W:(t + 1) * W])
            nt_ = st.tile([128, W], BF16, tag="nt")
            nc.scalar.activation(nt_[:, :], q[:, :], AF.Tanh)
            d_ = st.tile([128, W], BF16, tag="d")
            nc.vector.tensor_sub(d_[:, :], prev, nt_[:, :])
            e_ = st.tile([128, W], BF16, tag="e")
            nc.vector.tensor_mul(e_[:, :], d_[:, :], p_b[:, 0:W])
            nc.vector.tensor_add(outS[:, t * W:(t + 1) * W], nt_[:, :], e_[:, :])

            oc = min(64, TS)
            if (t + 1) % oc == 0:
                nc.sync.dma_start(d["outT"][:, (t + 1 - oc) * W:(t + 1) * W],
                                  outS[:, (t + 1 - oc) * W:(t + 1) * W])


def build_nc(TS: int = T, debug_dump: bool = False):
    nc = bacc.Bacc("TRN2", target_bir_lowering=False, debug=False)
    TB = TS * BS
    d = {}
    d["inT"] = nc.dram_tensor("inT", [I, TB], BF16, kind="ExternalInput").ap()
    d["h0T"] = nc.dram_tensor("h0T", [128, W], BF16, kind="ExternalInput").ap()
    d["whT"] = nc.dram_tensor("whT", [H, H], BF16, kind="ExternalInput").ap()
    d["wxT"] = nc.dram_tensor("wxT", [I, H], BF16, kind="ExternalInput").ap()
    d["zruT"] = nc.dram_tensor("zruT", [H, 2], BF16, kind="ExternalInput").ap()
    d["zrwT"] = nc.dram_tensor("zrwT", [I, 2], BF16, kind="ExternalInput").ap()
    d["hubT"] = nc.dram_tensor("hubT", [128, KC], F32, kind="ExternalInput").ap()
    d["hwbT"] = nc.dram_tensor("hwbT", [128, KC], F32, kind="ExternalInput").ap()
    d["gb"] = nc.dram_tensor("gb", [1, 2], BF16, kind="ExternalInput").ap()
    d["i2"] = nc.dram_tensor("i2", [2, 2], BF16, kind="ExternalInput").ap()
    d["sel2"] = nc.dram_tensor("sel2", [2, 256], BF16, kind="ExternalInput").ap()
    d["outT"] = nc.dram_tensor("outT", [128, TS * W], BF16, kind="ExternalOutput").ap()

    dbg = None
    if debug_dump:
        dbg = {
            "xnS": nc.dram_tensor("dbg_xnS", [128, TS * W], BF16, kind="ExternalOutput").ap(),
            "xzrS": nc.dram_tensor("dbg_xzrS", [1, TS * 2 * BS], BF16, kind="ExternalOutput").ap(),
            "p_q": nc.dram_tensor("dbg_p_q", [128, W], F32, kind="ExternalOutput").ap(),
            "p_b": nc.dram_tensor("dbg_p_b", [128, 2 * W], F32, kind="ExternalOutput").ap(),
            "zr_s": nc.dram_tensor("dbg_zr_s", [1, 2 * BS], BF16, kind="ExternalOutput").ap(),
            "q": nc.dram_tensor("dbg_q", [128, W], F32, kind="ExternalOutput").ap(),
            "nt": nc.dram_tensor("dbg_nt", [128, W], BF16, kind="ExternalOutput").ap(),
        }

    with tile.TileContext(nc) as tc, ExitStack() as ctx:
        _emit(ctx, tc, d, TS, dbg)
    nc.compile()
    return nc


def pack_inputs(inputs: dict, TS: int = T) -> list[dict]:
    """Host-side shard + relayout. Returns per-core in_maps."""
    f32 = np.float32
    inp = np.asarray(inputs["input"], f32)
    hid = np.asarray(inputs["hidden"], f32)
    wh = np.ascontiguousarray(np.asarray(inputs["h_u_w"], f32).T).astype(bf16)
    wx = np.ascontiguousarray(np.asarray(inputs["h_w_w"], f32).T).astype(bf16)
    zru = np.stack([np.asarray(inputs["zt_u_w"], f32)[0],
                    np.asarray(inputs["rt_u_w"], f32)[0]], axis=1).astype(bf16)
    zrw = np.stack([np.asarray(inputs["zt_w_w"], f32)[0],
                    np.asarray(inputs["rt_w_w"], f32)[0]], axis=1).astype(bf16)
    hub = np.ascontiguousarray(np.asarray(inputs["h_u_b"], f32).reshape(KC, 128).T)
    hwb = np.ascontiguousarray(np.asarray(inputs["h_w_b"], f32).reshape(KC, 128).T)
    gb = np.array([[float(inputs["zt_w_b"][0]) + float(inputs["zt_u_b"][0]),
                    float(inputs["rt_w_b"][0]) + float(inputs["rt_u_b"][0])]]).astype(bf16)
    i2 = np.eye(2, dtype=bf16)
    sel2 = np.zeros((2, 256), bf16)
    sel2[0, 0:128] = 1
    sel2[1, 128:256] = 1

    in_maps = []
    for c in range(NCORES):
        sl = inp[:TS, c * BS:(c + 1) * BS, :]                     # [TS, 8, I]
        inT = np.ascontiguousarray(sl.transpose(2, 0, 1).reshape(I, TS * BS)).astype(bf16)
        h0 = hid[c * BS:(c + 1) * BS, :]                          # [8, H]
        h0T = np.ascontiguousarray(
            h0.T.reshape(KC, 128, BS).transpose(1, 0, 2).reshape(128, W)).astype(bf16)
        in_maps.append({
            "inT": inT, "h0T": h0T, "whT": wh, "wxT": wx, "zruT": zru,
            "zrwT": zrw, "hubT": hub, "hwbT": hwb, "gb": gb, "i2": i2,
            "sel2": sel2,
        })
    return in_maps


def unpack_outputs(results: list[dict], TS: int = T):
    output = np.empty((TS, B, H), np.float32)
    for c in range(NCORES):
        o = np.asarray(results[c]["outT"]).reshape(128, TS, KC, BS)
        output[:, c * BS:(c + 1) * BS, :] = \
            o.transpose(1, 3, 2, 0).reshape(TS, BS, H).astype(np.float32)
    hidden_final = output[-1].copy()
    return output, hidden_final


_NC_CACHE = {}


def kernel(**inputs):
    if T not in _NC_CACHE:
        _NC_CACHE[T] = build_nc(T)
    nc = _NC_CACHE[T]
    in_maps = pack_inputs(inputs, T)
    res = run_bass_kernel_spmd(nc, in_maps, list(range(NCORES)))
    return unpack_outputs(res.results, T)


# revision 33
# speedup vs baseline: 1.2980x; 1.0930x over previous
"""GRUCell Trainium2 kernel: T=512, B=64, I=H=512, 8-way data parallel over B.

Strategy
--------
- Shard batch B=64 -> 8 rows per NeuronCore; weights replicated. No collectives.
- All recurrent state is kept in a transposed layout: a [128, 32] SBUF tile
  where element [p, c*8+b] = h[b, 128*c + p]  (c = H-chunk 0..3, b = local batch).
  This keeps H on partitions so every elementwise op streams only 32 columns.
- Per step t:
    * PE: psum_zr[2,8]  = sum_kc zru[kc].T @ h_kc  + I2.T @ xzr[t]   (gate preacts)
          psum_q[128,32] (4 col groups) = sum_kc Wh[kc,mc].T @ h_kc  (h_u matmul)
          psum_b[128,64] = ones.T @ zr_sig (broadcast gates across partitions)
    * ACT: zr_sig = sigmoid(psum_zr + gate_bias);  z_bcast copy; tanh
    * DVE: u = psum_q + h_u_b;  s = u * r_bcast;  q = s + xn[t];
           d = h - nt;  e = d * z_bcast;  h' = nt + e  -> written into outS slice
- xz/xr/xn input projections are precomputed on-device from a host-transposed
  input ([I, T*8] per core) with bf16 matmuls, fp32 accumulation.
- Output accumulates in SBUF ([128, T*32] bf16) and is DMA'd out in chunks;
  host code undoes the layout and returns ((T,B,H) float32, (B,H) float32).
"""

import numpy as np
import ml_dtypes
from contextlib import ExitStack

import concourse.bass as bass
import concourse.tile as tile
from concourse import bacc, mybir
from concourse.bass_utils import run_bass_kernel_spmd

AF = mybir.ActivationFunctionType

T, B, I, H = 512, 64, 512, 512
NCORES = 8
BS = B // NCORES          # 8 batch rows per core
KC = H // 128             # 4 partition chunks of the hidden dim
W = KC * BS               # 32 = width of one state slice
F32 = mybir.dt.float32
BF16 = mybir.dt.bfloat16
bf16 = ml_dtypes.bfloat16


def _emit(ctx: ExitStack, tc: "tile.TileContext", d: dict, TS: int, dbg: dict | None = None):
    nc = tc.nc
    TB = TS * BS           # flattened (t, b) count per core

    const = ctx.enter_context(tc.tile_pool(name="const", bufs=1))
    inS = const.tile([128, KC * TB], BF16, tag="inS")
    xnS = const.tile([128, TS * W], BF16, tag="xnS")
    outS = const.tile([128, TS * W], BF16, tag="outS")
    xzrS = const.tile([2, TB], BF16, tag="xzrS")   # row 0: xz, row 1: xr
    whS = const.tile([128, KC * H], BF16, tag="whS")
    wxS = const.tile([128, KC * H], BF16, tag="wxS")
    zruS = const.tile([128, KC * 2], BF16, tag="zruS")
    zrwS = const.tile([128, KC * 2], BF16, tag="zrwS")
    hubS = const.tile([128, KC], F32, tag="hubS")
    hwbS = const.tile([128, KC], F32, tag="hwbS")
    gbW = const.tile([1, 2], BF16, tag="gbW")
    onesS = const.tile([1, 512], BF16, tag="onesS")
    i2S = const.tile([2, 2], BF16, tag="i2S")
    selS = const.tile([2, 2 * 128], BF16, tag="selS")  # [:,0:128]=[1;0], [:,128:]=[0;1]
    h0S = const.tile([128, W], BF16, tag="h0S")
    nc.vector.memset(onesS[:, :], 1.0)
    nc.sync.dma_start(i2S[:, :], d["i2"][:, :])
    nc.sync.dma_start(selS[:, :], d["sel2"][:, :])

    for kc in range(KC):
        nc.sync.dma_start(inS[:, kc * TB:(kc + 1) * TB], d["inT"][kc * 128:(kc + 1) * 128, :])
        nc.sync.dma_start(whS[:, kc * H:(kc + 1) * H], d["whT"][kc * 128:(kc + 1) * 128, :])
        nc.sync.dma_start(wxS[:, kc * H:(kc + 1) * H], d["wxT"][kc * 128:(kc + 1) * 128, :])
        nc.sync.dma_start(zruS[:, kc * 2:(kc + 1) * 2], d["zruT"][kc * 128:(kc + 1) * 128, :])
        nc.sync.dma_start(zrwS[:, kc * 2:(kc + 1) * 2], d["zrwT"][kc * 128:(kc + 1) * 128, :])
    nc.sync.dma_start(hubS[:, :], d["hubT"][:, :])
    nc.sync.dma_start(hwbS[:, :], d["hwbT"][:, :])
    nc.sync.dma_start(gbW[:, :], d["gb"][:, :])
    nc.sync.dma_start(h0S[:, :], d["h0T"][:, :])

    # ---------------- input projections (xz/xr and xn), all t in parallel ----
    CT = min(512, TB)      # columns per projection tile
    NT = TB // CT
    with tc.tile_pool(name="ppq", bufs=2, space="PSUM") as ppq, \
         tc.tile_pool(name="ppz", bufs=2, space="PSUM") as ppz:
        for nt in range(NT):
            base = nt * CT
            pz = ppz.tile([2, CT], F32, tag="pz")
            for kc in range(KC):
                nc.tensor.matmul(pz[:, :], zrwS[:, 2 * kc:2 * kc + 2],
                                 inS[:, kc * TB + base: kc * TB + base + CT],
                                 start=(kc == 0), stop=False)
            # fold gate biases (w_b + u_b) in via a K=1 ones matmul
            nc.tensor.matmul(pz[:, :], gbW[:, :], onesS[0:1, 0:CT],
                             start=False, stop=True)
            nc.vector.tensor_copy(xzrS[:, base:base + CT], pz[:, :])
            for hc in range(KC):
                pq = ppq.tile([128, CT], F32, tag="pq")
                for ic in range(KC):
                    nc.tensor.matmul(pq[:, :], wxS[:, ic * H + hc * 128: ic * H + (hc + 1) * 128],
                                     inS[:, ic * TB + base: ic * TB + base + CT],
                                     start=(ic == 0), stop=(ic == KC - 1))
                # scatter into xnS[p, t*W + hc*BS + b] with per-partition bias h_w_b
                nrows = CT // BS
                t0 = base // BS
                xn_view = xnS[:, :].rearrange("p (t c b) -> p t c b", c=KC, b=BS)
                nc.vector.tensor_scalar_add(
                    xn_view[:, t0:t0 + nrows, hc, :],
                    pq[:, :].rearrange("p (t b) -> p t b", b=BS),
                    hwbS[:, hc:hc + 1])

    if dbg is not None:
        nc.sync.dma_start(dbg["xnS"], xnS[:, :])
        nc.sync.dma_start(dbg["xzrS"], xzrS[:, :])

    # ---------------- recurrence (single chain, latency-optimized) ---------
    with tc.tile_pool(name="st", bufs=3) as st, \
         tc.tile_pool(name="pqp", bufs=2, space="PSUM") as pqp, \
         tc.tile_pool(name="pzp", bufs=2, space="PSUM") as pzp, \
         tc.tile_pool(name="pbp", bufs=2, space="PSUM") as pbp:
        for t in range(TS):
            prev = h0S[:, :] if t == 0 else outS[:, (t - 1) * W: t * W]

            # gate preactivations: [2, BS] psum, z on partition 0, r on 1
            p_zr = pzp.tile([2, BS], F32, tag="p_zr")
            for kc in range(KC):
                nc.tensor.matmul(p_zr[:, :], zruS[:, 2 * kc:2 * kc + 2],
                                 prev[:, kc * BS:(kc + 1) * BS],
                                 start=(kc == 0), stop=False)
            nc.tensor.matmul(p_zr[:, :], i2S[:, :], xzrS[:, t * BS:(t + 1) * BS],
                             start=False, stop=True)

            zr_s = st.tile([2, BS], BF16, tag="zr")
            nc.scalar.activation(zr_s[:, :], p_zr[:, :], AF.Sigmoid)
            zm_s = st.tile([1, BS], BF16, tag="zm")
            nc.scalar.activation(zm_s[:, :], p_zr[0:1, :], AF.Sigmoid, scale=-1.0)

            # main h_u matmul, output transposed: [128(h), 4(c)*8(b)]
            p_q = pqp.tile([128, W], F32, tag="p_q")
            for mc in range(KC):
                for kc in range(KC):
                    nc.tensor.matmul(p_q[:, mc * BS:(mc + 1) * BS],
                                     whS[:, kc * H + mc * 128: kc * H + (mc + 1) * 128],
                                     prev[:, kc * BS:(kc + 1) * BS],
                                     start=(kc == 0), stop=(kc == KC - 1))

            # broadcast z, r, (1-z) across partitions via PE selector matmuls
            p_b = pbp.tile([128, 3 * W], F32, tag="p_b")
            rhs_zr = zr_s[:, :].unsqueeze(1).broadcast_to((2, KC, BS))
            rhs_zm = zm_s[:, :].unsqueeze(1).broadcast_to((1, KC, BS))
            pb_v = p_b[:, :].rearrange("p (g c b) -> p g c b", g=3, b=BS)
            nc.tensor.matmul(pb_v[:, 0, :, :], selS[:, 0:128], rhs_zr, start=True, stop=True)
            nc.tensor.matmul(pb_v[:, 1, :, :], selS[:, 128:256], rhs_zr, start=True, stop=True)
            nc.tensor.matmul(pb_v[:, 2, :, :], onesS[0:1, 0:128], rhs_zm, start=True, stop=True)

            u = st.tile([128, W], F32, tag="u")
            nc.vector.tensor_add(
                u[:, :].rearrange("p (c b) -> p c b", b=BS),
                p_q[:, :].rearrange("p (c b) -> p c b", b=BS),
                hubS[:, :].unsqueeze(2).broadcast_to((128, KC, BS)))
            s = st.tile([128, W], F32, tag="s")
            nc.vector.tensor_mul(s[:, :], u[:, :], p_b[:, W:2 * W])
            q = st.tile([128, W], F32, tag="q")
            nc.vector.tensor_add(q[:, :], s[:, :], xnS[:, t * W:(t + 1) * W])
            # z*prev runs in the tanh shadow on DVE
            e1 = st.tile([128, W], BF16, tag="e1")
            nc.vector.tensor_mul(e1[:, :], prev, p_b[:, 0:W])
            nt_ = st.tile([128, W], BF16, tag="nt")
            nc.scalar.activation(nt_[:, :], q[:, :], AF.Tanh)
            f_ = st.tile([128, W], BF16, tag="f")
            nc.vector.tensor_mul(f_[:, :], nt_[:, :], p_b[:, 2 * W:3 * W])
            nc.vector.tensor_add(outS[:, t * W:(t + 1) * W], e1[:, :], f_[:, :])

            oc = min(64, TS)
            if (t + 1) % oc == 0:
                nc.sync.dma_start(d["outT"][:, (t + 1 - oc) * W:(t + 1) * W],
                                  outS[:, (t + 1 - oc) * W:(t + 1) * W])


def build_nc(TS: int = T, debug_dump: bool = False):
    nc = bacc.Bacc("TRN2", target_bir_lowering=False, debug=False)
    TB = TS * BS
    d = {}
    d["inT"] = nc.dram_tensor("inT", [I, TB], BF16, kind="ExternalInput").ap()
    d["h0T"] = nc.dram_tensor("h0T", [128, W], BF16, kind="ExternalInput").ap()
    d["whT"] = nc.dram_tensor("whT", [H, H], BF16, kind="ExternalInput").ap()
    d["wxT"] = nc.dram_tensor("wxT", [I, H], BF16, kind="ExternalInput").ap()
    d["zruT"] = nc.dram_tensor("zruT", [H, 2], BF16, kind="ExternalInput").ap()
    d["zrwT"] = nc.dram_tensor("zrwT", [I, 2], BF16, kind="ExternalInput").ap()
    d["hubT"] = nc.dram_tensor("hubT", [128, KC], F32, kind="ExternalInput").ap()
    d["hwbT"] = nc.dram_tensor("hwbT", [128, KC], F32, kind="ExternalInput").ap()
    d["gb"] = nc.dram_tensor("gb", [1, 2], BF16, kind="ExternalInput").ap()
    d["i2"] = nc.dram_tensor("i2", [2, 2], BF16, kind="ExternalInput").ap()
    d["sel2"] = nc.dram_tensor("sel2", [2, 256], BF16, kind="ExternalInput").ap()
    d["outT"] = nc.dram_tensor("outT", [128, TS * W], BF16, kind="ExternalOutput").ap()

    dbg = None
    if debug_dump:
        dbg = {
            "xnS": nc.dram_tensor("dbg_xnS", [128, TS * W], BF16, kind="ExternalOutput").ap(),
            "xzrS": nc.dram_tensor("dbg_xzrS", [1, TS * 2 * BS], BF16, kind="ExternalOutput").ap(),
            "p_q": nc.dram_tensor("dbg_p_q", [128, W], F32, kind="ExternalOutput").ap(),
            "p_b": nc.dram_tensor("dbg_p_b", [128, 2 * W], F32, kind="ExternalOutput").ap(),
            "zr_s": nc.dram_tensor("dbg_zr_s", [1, 2 * BS], BF16, kind="ExternalOutput").ap(),
            "q": nc.dram_tensor("dbg_q", [128, W], F32, kind="ExternalOutput").ap(),
            "nt": nc.dram_tensor("dbg_nt", [128, W], BF16, kind="ExternalOutput").ap(),
        }

    with tile.TileContext(nc) as tc, ExitStack() as ctx:
        _emit(ctx, tc, d, TS, dbg)
    nc.compile()
    return nc


def pack_inputs(inputs: dict, TS: int = T) -> list[dict]:
    """Host-side shard + relayout. Returns per-core in_maps."""
    f32 = np.float32
    inp = np.asarray(inputs["input"], f32)
    hid = np.asarray(inputs["hidden"], f32)
    wh = np.ascontiguousarray(np.asarray(inputs["h_u_w"], f32).T).astype(bf16)
    wx = np.ascontiguousarray(np.asarray(inputs["h_w_w"], f32).T).astype(bf16)
    zru = np.stack([np.asarray(inputs["zt_u_w"], f32)[0],
                    np.asarray(inputs["rt_u_w"], f32)[0]], axis=1).astype(bf16)
    zrw = np.stack([np.asarray(inputs["zt_w_w"], f32)[0],
                    np.asarray(inputs["rt_w_w"], f32)[0]], axis=1).astype(bf16)
    hub = np.ascontiguousarray(np.asarray(inputs["h_u_b"], f32).reshape(KC, 128).T)
    hwb = np.ascontiguousarray(np.asarray(inputs["h_w_b"], f32).reshape(KC, 128).T)
    gb = np.array([[float(inputs["zt_w_b"][0]) + float(inputs["zt_u_b"][0]),
                    float(inputs["rt_w_b"][0]) + float(inputs["rt_u_b"][0])]]).astype(bf16)
    i2 = np.eye(2, dtype=bf16)
    sel2 = np.zeros((2, 256), bf16)
    sel2[0, 0:128] = 1
    sel2[1, 128:256] = 1

    in_maps = []
    for c in range(NCORES):
        sl = inp[:TS, c * BS:(c + 1) * BS, :]                     # [TS, 8, I]
        inT = np.ascontiguousarray(sl.transpose(2, 0, 1).reshape(I, TS * BS)).astype(bf16)
        h0 = hid[c * BS:(c + 1) * BS, :]                          # [8, H]
        h0T = np.ascontiguousarray(
            h0.T.reshape(KC, 128, BS).transpose(1, 0, 2).reshape(128, W)).astype(bf16)
        in_maps.append({
            "inT": inT, "h0T": h0T, "whT": wh, "wxT": wx, "zruT": zru,
            "zrwT": zrw, "hubT": hub, "hwbT": hwb, "gb": gb, "i2": i2,
            "sel2": sel2,
        })
    return in_maps


def unpack_outputs(results: list[dict], TS: int = T):
    output = np.empty((TS, B, H), np.float32)
    for c in range(NCORES):
        o = np.asarray(results[c]["outT"]).reshape(128, TS, KC, BS)
        output[:, c * BS:(c + 1) * BS, :] = \
            o.transpose(1, 3, 2, 0).reshape(TS, BS, H).astype(np.float32)
    hidden_final = output[-1].copy()
    return output, hidden_final


_NC_CACHE = {}


def kernel(**inputs):
    if T not in _NC_CACHE:
        _NC_CACHE[T] = build_nc(T)
    nc = _NC_CACHE[T]
    in_maps = pack_inputs(inputs, T)
    res = run_bass_kernel_spmd(nc, in_maps, list(range(NCORES)))
    return unpack_outputs(res.results, T)


# revision 37
# speedup vs baseline: 1.3040x; 1.0046x over previous
"""GRUCell Trainium2 kernel: T=512, B=64, I=H=512, 8-way data parallel over B.

Strategy
--------
- Shard batch B=64 -> 8 rows per NeuronCore; weights replicated. No collectives.
- All recurrent state is kept in a transposed layout: a [128, 32] SBUF tile
  where element [p, c*8+b] = h[b, 128*c + p]  (c = H-chunk 0..3, b = local batch).
  This keeps H on partitions so every elementwise op streams only 32 columns.
- Per step t:
    * PE: psum_zr[2,8]  = sum_kc zru[kc].T @ h_kc  + I2.T @ xzr[t]   (gate preacts)
          psum_q[128,32] (4 col groups) = sum_kc Wh[kc,mc].T @ h_kc  (h_u matmul)
          psum_b[128,64] = ones.T @ zr_sig (broadcast gates across partitions)
    * ACT: zr_sig = sigmoid(psum_zr + gate_bias);  z_bcast copy; tanh
    * DVE: u = psum_q + h_u_b;  s = u * r_bcast;  q = s + xn[t];
           d = h - nt;  e = d * z_bcast;  h' = nt + e  -> written into outS slice
- xz/xr/xn input projections are precomputed on-device from a host-transposed
  input ([I, T*8] per core) with bf16 matmuls, fp32 accumulation.
- Output accumulates in SBUF ([128, T*32] bf16) and is DMA'd out in chunks;
  host code undoes the layout and returns ((T,B,H) float32, (B,H) float32).
"""

import numpy as np
import ml_dtypes
from contextlib import ExitStack

import concourse.bass as bass
import concourse.tile as tile
from concourse import bacc, mybir
from concourse.bass_utils import run_bass_kernel_spmd

AF = mybir.ActivationFunctionType

T, B, I, H = 512, 64, 512, 512
NCORES = 8
BS = B // NCORES          # 8 batch rows per core
KC = H // 128             # 4 partition chunks of the hidden dim
W = KC * BS               # 32 = width of one state slice
F32 = mybir.dt.float32
BF16 = mybir.dt.bfloat16
bf16 = ml_dtypes.bfloat16


def _emit(ctx: ExitStack, tc: "tile.TileContext", d: dict, TS: int, dbg: dict | None = None):
    nc = tc.nc
    TB = TS * BS           # flattened (t, b) count per core

    const = ctx.enter_context(tc.tile_pool(name="const", bufs=1))
    inS = const.tile([128, KC * TB], BF16, tag="inS")
    xnS = const.tile([128, TS * W], BF16, tag="xnS")
    outS = const.tile([128, TS * W], BF16, tag="outS")
    xzrS = const.tile([2, TB], BF16, tag="xzrS")   # row 0: xz, row 1: xr
    whS = const.tile([128, KC * H], BF16, tag="whS")
    wxS = const.tile([128, KC * H], BF16, tag="wxS")
    zruS = const.tile([128, KC * 2], BF16, tag="zruS")
    zrwS = const.tile([128, KC * 2], BF16, tag="zrwS")
    hubS = const.tile([128, KC], F32, tag="hubS")
    hwbS = const.tile([128, KC], F32, tag="hwbS")
    gbW = const.tile([1, 2], BF16, tag="gbW")
    onesS = const.tile([1, 512], BF16, tag="onesS")
    i2S = const.tile([2, 2], BF16, tag="i2S")
    selS = const.tile([2, 2 * 128], BF16, tag="selS")  # [:,0:128]=[1;0], [:,128:]=[0;1]
    h0S = const.tile([128, W], BF16, tag="h0S")
    nc.vector.memset(onesS[:, :], 1.0)
    nc.sync.dma_start(i2S[:, :], d["i2"][:, :])
    nc.sync.dma_start(selS[:, :], d["sel2"][:, :])

    for kc in range(KC):
        nc.sync.dma_start(inS[:, kc * TB:(kc + 1) * TB], d["inT"][kc * 128:(kc + 1) * 128, :])
        nc.sync.dma_start(whS[:, kc * H:(kc + 1) * H], d["whT"][kc * 128:(kc + 1) * 128, :])
        nc.sync.dma_start(wxS[:, kc * H:(kc + 1) * H], d["wxT"][kc * 128:(kc + 1) * 128, :])
        nc.sync.dma_start(zruS[:, kc * 2:(kc + 1) * 2], d["zruT"][kc * 128:(kc + 1) * 128, :])
        nc.sync.dma_start(zrwS[:, kc * 2:(kc + 1) * 2], d["zrwT"][kc * 128:(kc + 1) * 128, :])
    nc.sync.dma_start(hubS[:, :], d["hubT"][:, :])
    nc.sync.dma_start(hwbS[:, :], d["hwbT"][:, :])
    nc.sync.dma_start(gbW[:, :], d["gb"][:, :])
    nc.sync.dma_start(h0S[:, :], d["h0T"][:, :])

    # ---------------- input projections (xz/xr and xn), all t in parallel ----
    CT = min(512, TB)      # columns per projection tile
    NT = TB // CT
    with tc.tile_pool(name="ppq", bufs=2, space="PSUM") as ppq, \
         tc.tile_pool(name="ppz", bufs=2, space="PSUM") as ppz:
        for nt in range(NT):
            base = nt * CT
            pz = ppz.tile([2, CT], F32, tag="pz")
            for kc in range(KC):
                nc.tensor.matmul(pz[:, :], zrwS[:, 2 * kc:2 * kc + 2],
                                 inS[:, kc * TB + base: kc * TB + base + CT],
                                 start=(kc == 0), stop=False)
            # fold gate biases (w_b + u_b) in via a K=1 ones matmul
            nc.tensor.matmul(pz[:, :], gbW[:, :], onesS[0:1, 0:CT],
                             start=False, stop=True)
            nc.vector.tensor_copy(xzrS[:, base:base + CT], pz[:, :])
            for hc in range(KC):
                pq = ppq.tile([128, CT], F32, tag="pq")
                for ic in range(KC):
                    nc.tensor.matmul(pq[:, :], wxS[:, ic * H + hc * 128: ic * H + (hc + 1) * 128],
                                     inS[:, ic * TB + base: ic * TB + base + CT],
                                     start=(ic == 0), stop=(ic == KC - 1))
                # scatter into xnS[p, t*W + hc*BS + b] with per-partition bias h_w_b
                nrows = CT // BS
                t0 = base // BS
                xn_view = xnS[:, :].rearrange("p (t c b) -> p t c b", c=KC, b=BS)
                nc.vector.tensor_scalar_add(
                    xn_view[:, t0:t0 + nrows, hc, :],
                    pq[:, :].rearrange("p (t b) -> p t b", b=BS),
                    hwbS[:, hc:hc + 1])

    if dbg is not None:
        nc.sync.dma_start(dbg["xnS"], xnS[:, :])
        nc.sync.dma_start(dbg["xzrS"], xzrS[:, :])

    # ---------------- recurrence (single chain, latency-optimized) ---------
    with tc.tile_pool(name="st", bufs=3) as st, \
         tc.tile_pool(name="pqp", bufs=2, space="PSUM") as pqp, \
         tc.tile_pool(name="pzp", bufs=2, space="PSUM") as pzp, \
         tc.tile_pool(name="pbp", bufs=1, space="PSUM") as pbp:
        for t in range(TS):
            prev = h0S[:, :] if t == 0 else outS[:, (t - 1) * W: t * W]

            # gate preactivations: [2, BS] psum, z on partition 0, r on 1
            p_zr = pzp.tile([2, BS], F32, tag="p_zr")
            for kc in range(KC):
                nc.tensor.matmul(p_zr[:, :], zruS[:, 2 * kc:2 * kc + 2],
                                 prev[:, kc * BS:(kc + 1) * BS],
                                 start=(kc == 0), stop=False)
            nc.tensor.matmul(p_zr[:, :], i2S[:, :], xzrS[:, t * BS:(t + 1) * BS],
                             start=False, stop=True)

            zr_s = st.tile([2, BS], BF16, tag="zr")
            nc.scalar.activation(zr_s[:, :], p_zr[:, :], AF.Sigmoid)
            zm_s = st.tile([1, BS], BF16, tag="zm")
            nc.scalar.activation(zm_s[:, :], p_zr[0:1, :], AF.Sigmoid, scale=-1.0)

            # main h_u matmul, output transposed: [128(h), 4(c)*8(b)]
            p_q = pqp.tile([128, W], F32, tag="p_q")
            for mc in range(KC):
                for kc in range(KC):
                    nc.tensor.matmul(p_q[:, mc * BS:(mc + 1) * BS],
                                     whS[:, kc * H + mc * 128: kc * H + (mc + 1) * 128],
                                     prev[:, kc * BS:(kc + 1) * BS],
                                     start=(kc == 0), stop=(kc == KC - 1))

            # broadcast z, r, (1-z) across partitions via PE selector matmuls.
            # Separate PSUM tiles so each consumer waits only its own matmul.
            rhs_zr = zr_s[:, :].unsqueeze(1).broadcast_to((2, KC, BS))
            rhs_zm = zm_s[:, :].unsqueeze(1).broadcast_to((1, KC, BS))
            rbP = pbp.tile([128, W], F32, tag="rbP")
            nc.tensor.matmul(rbP[:, :].rearrange("p (c b) -> p c b", b=BS),
                             selS[:, 128:256], rhs_zr, start=True, stop=True)
            zbP = pbp.tile([128, W], F32, tag="zbP")
            nc.tensor.matmul(zbP[:, :].rearrange("p (c b) -> p c b", b=BS),
                             selS[:, 0:128], rhs_zr, start=True, stop=True)
            zmP = pbp.tile([128, W], F32, tag="zmP")
            nc.tensor.matmul(zmP[:, :].rearrange("p (c b) -> p c b", b=BS),
                             onesS[0:1, 0:128], rhs_zm, start=True, stop=True)

            u = st.tile([128, W], F32, tag="u")
            nc.vector.tensor_add(
                u[:, :].rearrange("p (c b) -> p c b", b=BS),
                p_q[:, :].rearrange("p (c b) -> p c b", b=BS),
                hubS[:, :].unsqueeze(2).broadcast_to((128, KC, BS)))
            s = st.tile([128, W], F32, tag="s")
            nc.vector.tensor_mul(s[:, :], u[:, :], rbP[:, :])
            q = st.tile([128, W], F32, tag="q")
            nc.vector.tensor_add(q[:, :], s[:, :], xnS[:, t * W:(t + 1) * W])
            # z*prev runs in the tanh shadow on DVE
            e1 = st.tile([128, W], BF16, tag="e1")
            nc.vector.tensor_mul(e1[:, :], prev, zbP[:, :])
            nt_ = st.tile([128, W], BF16, tag="nt")
            nc.scalar.activation(nt_[:, :], q[:, :], AF.Tanh)
            f_ = st.tile([128, W], BF16, tag="f")
            nc.vector.tensor_mul(f_[:, :], nt_[:, :], zmP[:, :])
            nc.vector.tensor_add(outS[:, t * W:(t + 1) * W], e1[:, :], f_[:, :])

            oc = min(64, TS)
            if (t + 1) % oc == 0:
                nc.sync.dma_start(d["outT"][:, (t + 1 - oc) * W:(t + 1) * W],
                                  outS[:, (t + 1 - oc) * W:(t + 1) * W])


def build_nc(TS: int = T, debug_dump: bool = False):
    nc = bacc.Bacc("TRN2", target_bir_lowering=False, debug=False)
    TB = TS * BS
    d = {}
    d["inT"] = nc.dram_tensor("inT", [I, TB], BF16, kind="ExternalInput").ap()
    d["h0T"] = nc.dram_tensor("h0T", [128, W], BF16, kind="ExternalInput").ap()
    d["whT"] = nc.dram_tensor("whT", [H, H], BF16, kind="ExternalInput").ap()
    d["wxT"] = nc.dram_tensor("wxT", [I, H], BF16, kind="ExternalInput").ap()
    d["zruT"] = nc.dram_tensor("zruT", [H, 2], BF16, kind="ExternalInput").ap()
    d["zrwT"] = nc.dram_tensor("zrwT", [I, 2], BF16, kind="ExternalInput").ap()
    d["hubT"] = nc.dram_tensor("hubT", [128, KC], F32, kind="ExternalInput").ap()
    d["hwbT"] = nc.dram_tensor("hwbT", [128, KC], F32, kind="ExternalInput").ap()
    d["gb"] = nc.dram_tensor("gb", [1, 2], BF16, kind="ExternalInput").ap()
    d["i2"] = nc.dram_tensor("i2", [2, 2], BF16, kind="ExternalInput").ap()
    d["sel2"] = nc.dram_tensor("sel2", [2, 256], BF16, kind="ExternalInput").ap()
    d["outT"] = nc.dram_tensor("outT", [128, TS * W], BF16, kind="ExternalOutput").ap()

    dbg = None
    if debug_dump:
        dbg = {
            "xnS": nc.dram_tensor("dbg_xnS", [128, TS * W], BF16, kind="ExternalOutput").ap(),
            "xzrS": nc.dram_tensor("dbg_xzrS", [1, TS * 2 * BS], BF16, kind="ExternalOutput").ap(),
            "p_q": nc.dram_tensor("dbg_p_q", [128, W], F32, kind="ExternalOutput").ap(),
            "p_b": nc.dram_tensor("dbg_p_b", [128, 2 * W], F32, kind="ExternalOutput").ap(),
            "zr_s": nc.dram_tensor("dbg_zr_s", [1, 2 * BS], BF16, kind="ExternalOutput").ap(),
            "q": nc.dram_tensor("dbg_q", [128, W], F32, kind="ExternalOutput").ap(),
            "nt": nc.dram_tensor("dbg_nt", [128, W], BF16, kind="ExternalOutput").ap(),
        }

    with tile.TileContext(nc) as tc, ExitStack() as ctx:
        _emit(ctx, tc, d, TS, dbg)
    nc.compile()
    return nc


def pack_inputs(inputs: dict, TS: int = T) -> list[dict]:
    """Host-side shard + relayout. Returns per-core in_maps."""
    f32 = np.float32
    inp = np.asarray(inputs["input"], f32)
    hid = np.asarray(inputs["hidden"], f32)
    wh = np.ascontiguousarray(np.asarray(inputs["h_u_w"], f32).T).astype(bf16)
    wx = np.ascontiguousarray(np.asarray(inputs["h_w_w"], f32).T).astype(bf16)
    zru = np.stack([np.asarray(inputs["zt_u_w"], f32)[0],
                    np.asarray(inputs["rt_u_w"], f32)[0]], axis=1).astype(bf16)
    zrw = np.stack([np.asarray(inputs["zt_w_w"], f32)[0],
                    np.asarray(inputs["rt_w_w"], f32)[0]], axis=1).astype(bf16)
    hub = np.ascontiguousarray(np.asarray(inputs["h_u_b"], f32).reshape(KC, 128).T)
    hwb = np.ascontiguousarray(np.asarray(inputs["h_w_b"], f32).reshape(KC, 128).T)
    gb = np.array([[float(inputs["zt_w_b"][0]) + float(inputs["zt_u_b"][0]),
                    float(inputs["rt_w_b"][0]) + float(inputs["rt_u_b"][0])]]).astype(bf16)
    i2 = np.eye(2, dtype=bf16)
    sel2 = np.zeros((2, 256), bf16)
    sel2[0, 0:128] = 1
    sel2[1, 128:256] = 1

    in_maps = []
    for c in range(NCORES):
        sl = inp[:TS, c * BS:(c + 1) * BS, :]                     # [TS, 8, I]
        inT = np.ascontiguousarray(sl.transpose(2, 0, 1).reshape(I, TS * BS)).astype(bf16)
        h0 = hid[c * BS:(c + 1) * BS, :]                          # [8, H]
        h0T = np.ascontiguousarray(
            h0.T.reshape(KC, 128, BS).transpose(1, 0, 2).reshape(128, W)).astype(bf16)
        in_maps.append({
            "inT": inT, "h0T": h0T, "whT": wh, "wxT": wx, "zruT": zru,
            "zrwT": zrw, "hubT": hub, "hwbT": hwb, "gb": gb, "i2": i2,
            "sel2": sel2,
        })
    return in_maps


def unpack_outputs(results: list[dict], TS: int = T):
    output = np.empty((TS, B, H), np.float32)
    for c in range(NCORES):
        o = np.asarray(results[c]["outT"]).reshape(128, TS, KC, BS)
        output[:, c * BS:(c + 1) * BS, :] = \
            o.transpose(1, 3, 2, 0).reshape(TS, BS, H).astype(np.float32)
    hidden_final = output[-1].copy()
    return output, hidden_final


_NC_CACHE = {}


def kernel(**inputs):
    if T not in _NC_CACHE:
        _NC_CACHE[T] = build_nc(T)
    nc = _NC_CACHE[T]
    in_maps = pack_inputs(inputs, T)
    res = run_bass_kernel_spmd(nc, in_maps, list(range(NCORES)))
    return unpack_outputs(res.results, T)


# revision 41
# speedup vs baseline: 1.4057x; 1.0780x over previous
"""GRUCell Trainium2 kernel: T=512, B=64, I=H=512, 8-way data parallel over B.

Strategy
--------
- Shard batch B=64 -> 8 rows per NeuronCore; weights replicated. No collectives.
- All recurrent state is kept in a transposed layout: a [128, 32] SBUF tile
  where element [p, c*8+b] = h[b, 128*c + p]  (c = H-chunk 0..3, b = local batch).
  This keeps H on partitions so every elementwise op streams only 32 columns.
- Per step t:
    * PE: psum_zr[2,8]  = sum_kc zru[kc].T @ h_kc  + I2.T @ xzr[t]   (gate preacts)
          psum_q[128,32] (4 col groups) = sum_kc Wh[kc,mc].T @ h_kc  (h_u matmul)
          psum_b[128,64] = ones.T @ zr_sig (broadcast gates across partitions)
    * ACT: zr_sig = sigmoid(psum_zr + gate_bias);  z_bcast copy; tanh
    * DVE: u = psum_q + h_u_b;  s = u * r_bcast;  q = s + xn[t];
           d = h - nt;  e = d * z_bcast;  h' = nt + e  -> written into outS slice
- xz/xr/xn input projections are precomputed on-device from a host-transposed
  input ([I, T*8] per core) with bf16 matmuls, fp32 accumulation.
- Output accumulates in SBUF ([128, T*32] bf16) and is DMA'd out in chunks;
  host code undoes the layout and returns ((T,B,H) float32, (B,H) float32).
"""

import numpy as np
import ml_dtypes
from contextlib import ExitStack

import concourse.bass as bass
import concourse.tile as tile
from concourse import bacc, mybir
from concourse.bass_utils import run_bass_kernel_spmd

AF = mybir.ActivationFunctionType

T, B, I, H = 512, 64, 512, 512
NCORES = 8
BS = B // NCORES          # 8 batch rows per core
KC = H // 128             # 4 partition chunks of the hidden dim
W = KC * BS               # 32 = width of one state slice
F32 = mybir.dt.float32
BF16 = mybir.dt.bfloat16
bf16 = ml_dtypes.bfloat16


def _emit(ctx: ExitStack, tc: "tile.TileContext", d: dict, TS: int, dbg: dict | None = None):
    nc = tc.nc
    TB = TS * BS           # flattened (t, b) count per core

    const = ctx.enter_context(tc.tile_pool(name="const", bufs=1))
    inS = const.tile([128, KC * TB], BF16, tag="inS")
    xnS = const.tile([128, TS * W], BF16, tag="xnS")
    outS = const.tile([128, TS * W], BF16, tag="outS")
    xzrS = const.tile([2, TB], BF16, tag="xzrS")   # row 0: xz, row 1: xr
    whS = const.tile([128, KC * H], BF16, tag="whS")
    wxS = const.tile([128, KC * H], BF16, tag="wxS")
    zruS = const.tile([128, KC * 2], BF16, tag="zruS")
    zrwS = const.tile([128, KC * 2], BF16, tag="zrwS")
    hubS = const.tile([128, KC], F32, tag="hubS")
    hwbS = const.tile([128, KC], F32, tag="hwbS")
    gbW = const.tile([1, 2], BF16, tag="gbW")
    onesS = const.tile([1, 512], BF16, tag="onesS")
    i2S = const.tile([2, 2], BF16, tag="i2S")
    selS = const.tile([2, 2 * 128], BF16, tag="selS")  # [:,0:128]=[1;0], [:,128:]=[0;1]
    h0S = const.tile([128, W], BF16, tag="h0S")
    nc.vector.memset(onesS[:, :], 1.0)
    nc.sync.dma_start(i2S[:, :], d["i2"][:, :])
    nc.sync.dma_start(selS[:, :], d["sel2"][:, :])

    for kc in range(KC):
        nc.sync.dma_start(inS[:, kc * TB:(kc + 1) * TB], d["inT"][kc * 128:(kc + 1) * 128, :])
        nc.sync.dma_start(whS[:, kc * H:(kc + 1) * H], d["whT"][kc * 128:(kc + 1) * 128, :])
        nc.sync.dma_start(wxS[:, kc * H:(kc + 1) * H], d["wxT"][kc * 128:(kc + 1) * 128, :])
        nc.sync.dma_start(zruS[:, kc * 2:(kc + 1) * 2], d["zruT"][kc * 128:(kc + 1) * 128, :])
        nc.sync.dma_start(zrwS[:, kc * 2:(kc + 1) * 2], d["zrwT"][kc * 128:(kc + 1) * 128, :])
    nc.sync.dma_start(hubS[:, :], d["hubT"][:, :])
    nc.sync.dma_start(hwbS[:, :], d["hwbT"][:, :])
    nc.sync.dma_start(gbW[:, :], d["gb"][:, :])
    nc.sync.dma_start(h0S[:, :], d["h0T"][:, :])

    # ---------------- input projections (xz/xr and xn), all t in parallel ----
    CT = min(512, TB)      # columns per projection tile
    NT = TB // CT
    with tc.tile_pool(name="ppq", bufs=2, space="PSUM") as ppq, \
         tc.tile_pool(name="ppz", bufs=2, space="PSUM") as ppz:
        for nt in range(NT):
            base = nt * CT
            pz = ppz.tile([2, CT], F32, tag="pz")
            for kc in range(KC):
                nc.tensor.matmul(pz[:, :], zrwS[:, 2 * kc:2 * kc + 2],
                                 inS[:, kc * TB + base: kc * TB + base + CT],
                                 start=(kc == 0), stop=False)
            # fold gate biases (w_b + u_b) in via a K=1 ones matmul
            nc.tensor.matmul(pz[:, :], gbW[:, :], onesS[0:1, 0:CT],
                             start=False, stop=True)
            nc.vector.tensor_copy(xzrS[:, base:base + CT], pz[:, :])
            for hc in range(KC):
                pq = ppq.tile([128, CT], F32, tag="pq")
                for ic in range(KC):
                    nc.tensor.matmul(pq[:, :], wxS[:, ic * H + hc * 128: ic * H + (hc + 1) * 128],
                                     inS[:, ic * TB + base: ic * TB + base + CT],
                                     start=(ic == 0), stop=(ic == KC - 1))
                # scatter into xnS[p, t*W + hc*BS + b] with per-partition bias h_w_b
                nrows = CT // BS
                t0 = base // BS
                xn_view = xnS[:, :].rearrange("p (t c b) -> p t c b", c=KC, b=BS)
                nc.vector.tensor_scalar_add(
                    xn_view[:, t0:t0 + nrows, hc, :],
                    pq[:, :].rearrange("p (t b) -> p t b", b=BS),
                    hwbS[:, hc:hc + 1])

    if dbg is not None:
        nc.sync.dma_start(dbg["xnS"], xnS[:, :])
        nc.sync.dma_start(dbg["xzrS"], xzrS[:, :])

    # ---------------- recurrence (single chain, latency-optimized) ---------
    # p_q is padded so each mc accumulation region sits in its own 2KB PSUM
    # bank (zero region) — all four groups stay open concurrently across the
    # e1/f split matmuls.
    QS = 512
    with tc.tile_pool(name="st", bufs=3) as st, \
         tc.tile_pool(name="pqp", bufs=1, space="PSUM") as pqp, \
         tc.tile_pool(name="pzp", bufs=1, space="PSUM") as pzp, \
         tc.tile_pool(name="pbp", bufs=1, space="PSUM") as pbp:
        e1_prev = f_prev = None
        for t in range(TS):
            prev = h0S[:, :] if t == 0 else outS[:, (t - 1) * W: t * W]

            # h(t) = e1_prev + f_prev, so every matmul against h(t) is split
            # into an e1 part (runs in the previous tanh's shadow) and an f
            # part (runs as soon as f lands) — the h' add is output-only.
            p_zr = pzp.tile([2, BS], F32, tag="p_zr")
            p_q = pqp.tile([128, KC * QS], F32, tag="p_q")
            if t == 0:
                for kc in range(KC):
                    nc.tensor.matmul(p_zr[:, :], zruS[:, 2 * kc:2 * kc + 2],
                                     prev[:, kc * BS:(kc + 1) * BS],
                                     start=(kc == 0), stop=False)
                nc.tensor.matmul(p_zr[:, :], i2S[:, :], xzrS[:, t * BS:(t + 1) * BS],
                                 start=False, stop=True)
                for mc in range(KC):
                    for kc in range(KC):
                        nc.tensor.matmul(p_q[:, mc * QS: mc * QS + BS],
                                         whS[:, kc * H + mc * 128: kc * H + (mc + 1) * 128],
                                         prev[:, kc * BS:(kc + 1) * BS],
                                         start=(kc == 0), stop=(kc == KC - 1))
            else:
                # e1-part: gates then main (PE busy during tanh shadow)
                for kc in range(KC):
                    nc.tensor.matmul(p_zr[:, :], zruS[:, 2 * kc:2 * kc + 2],
                                     e1_prev[:, kc * BS:(kc + 1) * BS],
                                     start=(kc == 0), stop=False)
                for mc in range(KC):
                    for kc in range(KC):
                        nc.tensor.matmul(p_q[:, mc * QS: mc * QS + BS],
                                         whS[:, kc * H + mc * 128: kc * H + (mc + 1) * 128],
                                         e1_prev[:, kc * BS:(kc + 1) * BS],
                                         start=(kc == 0), stop=False)
                # f-part: gates (+x injection) then main
                for kc in range(KC):
                    nc.tensor.matmul(p_zr[:, :], zruS[:, 2 * kc:2 * kc + 2],
                                     f_prev[:, kc * BS:(kc + 1) * BS],
                                     start=False, stop=False)
                nc.tensor.matmul(p_zr[:, :], i2S[:, :], xzrS[:, t * BS:(t + 1) * BS],
                                 start=False, stop=True)
                for mc in range(KC):
                    for kc in range(KC):
                        nc.tensor.matmul(p_q[:, mc * QS: mc * QS + BS],
                                         whS[:, kc * H + mc * 128: kc * H + (mc + 1) * 128],
                                         f_prev[:, kc * BS:(kc + 1) * BS],
                                         start=False, stop=(kc == KC - 1))

            zr_s = st.tile([2, BS], BF16, tag="zr")
            nc.scalar.activation(zr_s[:, :], p_zr[:, :], AF.Sigmoid)
            zm_s = st.tile([1, BS], BF16, tag="zm")
            nc.scalar.activation(zm_s[:, :], p_zr[0:1, :], AF.Sigmoid, scale=-1.0)

            # broadcast z, r, (1-z) across partitions via PE selector matmuls.
            # Separate PSUM tiles so each consumer waits only its own matmul.
            rhs_zr = zr_s[:, :].unsqueeze(1).broadcast_to((2, KC, BS))
            rhs_zm = zm_s[:, :].unsqueeze(1).broadcast_to((1, KC, BS))
            rbP = pbp.tile([128, W], F32, tag="rbP")
            nc.tensor.matmul(rbP[:, :].rearrange("p (c b) -> p c b", b=BS),
                             selS[:, 128:256], rhs_zr, start=True, stop=True)
            zbP = pbp.tile([128, W], F32, tag="zbP")
            nc.tensor.matmul(zbP[:, :].rearrange("p (c b) -> p c b", b=BS),
                             selS[:, 0:128], rhs_zr, start=True, stop=True)
            zmP = pbp.tile([128, W], F32, tag="zmP")
            nc.tensor.matmul(zmP[:, :].rearrange("p (c b) -> p c b", b=BS),
                             onesS[0:1, 0:128], rhs_zm, start=True, stop=True)

            u = st.tile([128, W], F32, tag="u")
            nc.vector.tensor_add(
                u[:, :].rearrange("p (c b) -> p c b", b=BS),
                p_q[:, :].rearrange("p (c q) -> p c q", q=QS)[:, :, 0:BS],
                hubS[:, :].unsqueeze(2).broadcast_to((128, KC, BS)))
            s = st.tile([128, W], F32, tag="s")
            nc.vector.tensor_mul(s[:, :], u[:, :], rbP[:, :])
            q = st.tile([128, W], F32, tag="q")
            nc.vector.tensor_add(q[:, :], s[:, :], xnS[:, t * W:(t + 1) * W])
            # z*prev runs in the tanh shadow on DVE
            e1 = st.tile([128, W], BF16, tag="e1")
            nc.vector.tensor_mul(e1[:, :], prev, zbP[:, :])
            nt_ = st.tile([128, W], BF16, tag="nt")
            nc.scalar.activation(nt_[:, :], q[:, :], AF.Tanh)
            f_ = st.tile([128, W], BF16, tag="f")
            nc.vector.tensor_mul(f_[:, :], nt_[:, :], zmP[:, :])
            nc.vector.tensor_add(outS[:, t * W:(t + 1) * W], e1[:, :], f_[:, :])
            e1_prev, f_prev = e1, f_

            oc = min(64, TS)
            if (t + 1) % oc == 0:
                nc.sync.dma_start(d["outT"][:, (t + 1 - oc) * W:(t + 1) * W],
                                  outS[:, (t + 1 - oc) * W:(t + 1) * W])


def build_nc(TS: int = T, debug_dump: bool = False):
    nc = bacc.Bacc("TRN2", target_bir_lowering=False, debug=False)
    TB = TS * BS
    d = {}
    d["inT"] = nc.dram_tensor("inT", [I, TB], BF16, kind="ExternalInput").ap()
    d["h0T"] = nc.dram_tensor("h0T", [128, W], BF16, kind="ExternalInput").ap()
    d["whT"] = nc.dram_tensor("whT", [H, H], BF16, kind="ExternalInput").ap()
    d["wxT"] = nc.dram_tensor("wxT", [I, H], BF16, kind="ExternalInput").ap()
    d["zruT"] = nc.dram_tensor("zruT", [H, 2], BF16, kind="ExternalInput").ap()
    d["zrwT"] = nc.dram_tensor("zrwT", [I, 2], BF16, kind="ExternalInput").ap()
    d["hubT"] = nc.dram_tensor("hubT", [128, KC], F32, kind="ExternalInput").ap()
    d["hwbT"] = nc.dram_tensor("hwbT", [128, KC], F32, kind="ExternalInput").ap()
    d["gb"] = nc.dram_tensor("gb", [1, 2], BF16, kind="ExternalInput").ap()
    d["i2"] = nc.dram_tensor("i2", [2, 2], BF16, kind="ExternalInput").ap()
    d["sel2"] = nc.dram_tensor("sel2", [2, 256], BF16, kind="ExternalInput").ap()
    d["outT"] = nc.dram_tensor("outT", [128, TS * W], BF16, kind="ExternalOutput").ap()

    dbg = None
    if debug_dump:
        dbg = {
            "xnS": nc.dram_tensor("dbg_xnS", [128, TS * W], BF16, kind="ExternalOutput").ap(),
            "xzrS": nc.dram_tensor("dbg_xzrS", [1, TS * 2 * BS], BF16, kind="ExternalOutput").ap(),
            "p_q": nc.dram_tensor("dbg_p_q", [128, W], F32, kind="ExternalOutput").ap(),
            "p_b": nc.dram_tensor("dbg_p_b", [128, 2 * W], F32, kind="ExternalOutput").ap(),
            "zr_s": nc.dram_tensor("dbg_zr_s", [1, 2 * BS], BF16, kind="ExternalOutput").ap(),
            "q": nc.dram_tensor("dbg_q", [128, W], F32, kind="ExternalOutput").ap(),
            "nt": nc.dram_tensor("dbg_nt", [128, W], BF16, kind="ExternalOutput").ap(),
        }

    with tile.TileContext(nc) as tc, ExitStack() as ctx:
        _emit(ctx, tc, d, TS, dbg)
    nc.compile()
    return nc


def pack_inputs(inputs: dict, TS: int = T) -> list[dict]:
    """Host-side shard + relayout. Returns per-core in_maps."""
    f32 = np.float32
    inp = np.asarray(inputs["input"], f32)
    hid = np.asarray(inputs["hidden"], f32)
    wh = np.ascontiguousarray(np.asarray(inputs["h_u_w"], f32).T).astype(bf16)
    wx = np.ascontiguousarray(np.asarray(inputs["h_w_w"], f32).T).astype(bf16)
    zru = np.stack([np.asarray(inputs["zt_u_w"], f32)[0],
                    np.asarray(inputs["rt_u_w"], f32)[0]], axis=1).astype(bf16)
    zrw = np.stack([np.asarray(inputs["zt_w_w"], f32)[0],
                    np.asarray(inputs["rt_w_w"], f32)[0]], axis=1).astype(bf16)
    hub = np.ascontiguousarray(np.asarray(inputs["h_u_b"], f32).reshape(KC, 128).T)
    hwb = np.ascontiguousarray(np.asarray(inputs["h_w_b"], f32).reshape(KC, 128).T)
    gb = np.array([[float(inputs["zt_w_b"][0]) + float(inputs["zt_u_b"][0]),
                    float(inputs["rt_w_b"][0]) + float(inputs["rt_u_b"][0])]]).astype(bf16)
    i2 = np.eye(2, dtype=bf16)
    sel2 = np.zeros((2, 256), bf16)
    sel2[0, 0:128] = 1
    sel2[1, 128:256] = 1

    in_maps = []
    for c in range(NCORES):
        sl = inp[:TS, c * BS:(c + 1) * BS, :]                     # [TS, 8, I]
        inT = np.ascontiguousarray(sl.transpose(2, 0, 1).reshape(I, TS * BS)).astype(bf16)
        h0 = hid[c * BS:(c + 1) * BS, :]                          # [8, H]
        h0T = np.ascontiguousarray(
            h0.T.reshape(KC, 128, BS).transpose(1, 0, 2).reshape(128, W)).astype(bf16)
        in_maps.append({
            "inT": inT, "h0T": h0T, "whT": wh, "wxT": wx, "zruT": zru,
            "zrwT": zrw, "hubT": hub, "hwbT": hwb, "gb": gb, "i2": i2,
            "sel2": sel2,
        })
    return in_maps


def unpack_outputs(results: list[dict], TS: int = T):
    output = np.empty((TS, B, H), np.float32)
    for c in range(NCORES):
        o = np.asarray(results[c]["outT"]).reshape(128, TS, KC, BS)
        output[:, c * BS:(c + 1) * BS, :] = \
            o.transpose(1, 3, 2, 0).reshape(TS, BS, H).astype(np.float32)
    hidden_final = output[-1].copy()
    return output, hidden_final


_NC_CACHE = {}


def kernel(**inputs):
    if T not in _NC_CACHE:
        _NC_CACHE[T] = build_nc(T)
    nc = _NC_CACHE[T]
    in_maps = pack_inputs(inputs, T)
    res = run_bass_kernel_spmd(nc, in_maps, list(range(NCORES)))
    return unpack_outputs(res.results, T)


# revision 42
# speedup vs baseline: 1.4656x; 1.0426x over previous
"""GRUCell Trainium2 kernel: T=512, B=64, I=H=512, 8-way data parallel over B.

Strategy
--------
- Shard batch B=64 -> 8 rows per NeuronCore; weights replicated. No collectives.
- All recurrent state is kept in a transposed layout: a [128, 32] SBUF tile
  where element [p, c*8+b] = h[b, 128*c + p]  (c = H-chunk 0..3, b = local batch).
  This keeps H on partitions so every elementwise op streams only 32 columns.
- Per step t:
    * PE: psum_zr[2,8]  = sum_kc zru[kc].T @ h_kc  + I2.T @ xzr[t]   (gate preacts)
          psum_q[128,32] (4 col groups) = sum_kc Wh[kc,mc].T @ h_kc  (h_u matmul)
          psum_b[128,64] = ones.T @ zr_sig (broadcast gates across partitions)
    * ACT: zr_sig = sigmoid(psum_zr + gate_bias);  z_bcast copy; tanh
    * DVE: u = psum_q + h_u_b;  s = u * r_bcast;  q = s + xn[t];
           d = h - nt;  e = d * z_bcast;  h' = nt + e  -> written into outS slice
- xz/xr/xn input projections are precomputed on-device from a host-transposed
  input ([I, T*8] per core) with bf16 matmuls, fp32 accumulation.
- Output accumulates in SBUF ([128, T*32] bf16) and is DMA'd out in chunks;
  host code undoes the layout and returns ((T,B,H) float32, (B,H) float32).
"""

import numpy as np
import ml_dtypes
from contextlib import ExitStack

import concourse.bass as bass
import concourse.tile as tile
from concourse import bacc, mybir
from concourse.bass_utils import run_bass_kernel_spmd

AF = mybir.ActivationFunctionType

T, B, I, H = 512, 64, 512, 512
NCORES = 8
BS = B // NCORES          # 8 batch rows per core
KC = H // 128             # 4 partition chunks of the hidden dim
W = KC * BS               # 32 = width of one state slice
F32 = mybir.dt.float32
BF16 = mybir.dt.bfloat16
bf16 = ml_dtypes.bfloat16


def _emit(ctx: ExitStack, tc: "tile.TileContext", d: dict, TS: int, dbg: dict | None = None):
    nc = tc.nc
    TB = TS * BS           # flattened (t, b) count per core

    const = ctx.enter_context(tc.tile_pool(name="const", bufs=1))
    inS = const.tile([128, KC * TB], BF16, tag="inS")
    xnS = const.tile([128, TS * W], BF16, tag="xnS")
    outS = const.tile([128, TS * W], BF16, tag="outS")
    xzrS = const.tile([2, TB], BF16, tag="xzrS")   # row 0: xz, row 1: xr
    whS = const.tile([128, KC * H], BF16, tag="whS")
    wxS = const.tile([128, KC * H], BF16, tag="wxS")
    zruS = const.tile([128, KC * 2], BF16, tag="zruS")
    zrwS = const.tile([128, KC * 2], BF16, tag="zrwS")
    hubS = const.tile([128, KC], F32, tag="hubS")
    hwbS = const.tile([128, KC], F32, tag="hwbS")
    gbW = const.tile([1, 2], BF16, tag="gbW")
    onesS = const.tile([1, 512], BF16, tag="onesS")
    i2S = const.tile([2, 2], BF16, tag="i2S")
    selS = const.tile([2, 2 * 128], BF16, tag="selS")  # [:,0:128]=[1;0], [:,128:]=[0;1]
    h0S = const.tile([128, W], BF16, tag="h0S")
    nc.vector.memset(onesS[:, :], 1.0)
    nc.sync.dma_start(i2S[:, :], d["i2"][:, :])
    nc.sync.dma_start(selS[:, :], d["sel2"][:, :])

    for kc in range(KC):
        nc.sync.dma_start(inS[:, kc * TB:(kc + 1) * TB], d["inT"][kc * 128:(kc + 1) * 128, :])
        nc.sync.dma_start(whS[:, kc * H:(kc + 1) * H], d["whT"][kc * 128:(kc + 1) * 128, :])
        nc.sync.dma_start(wxS[:, kc * H:(kc + 1) * H], d["wxT"][kc * 128:(kc + 1) * 128, :])
        nc.sync.dma_start(zruS[:, kc * 2:(kc + 1) * 2], d["zruT"][kc * 128:(kc + 1) * 128, :])
        nc.sync.dma_start(zrwS[:, kc * 2:(kc + 1) * 2], d["zrwT"][kc * 128:(kc + 1) * 128, :])
    nc.sync.dma_start(hubS[:, :], d["hubT"][:, :])
    nc.sync.dma_start(hwbS[:, :], d["hwbT"][:, :])
    nc.sync.dma_start(gbW[:, :], d["gb"][:, :])
    nc.sync.dma_start(h0S[:, :], d["h0T"][:, :])

    # ---------------- input projections (xz/xr and xn), all t in parallel ----
    CT = min(512, TB)      # columns per projection tile
    NT = TB // CT
    with tc.tile_pool(name="ppq", bufs=2, space="PSUM") as ppq, \
         tc.tile_pool(name="ppz", bufs=2, space="PSUM") as ppz:
        for nt in range(NT):
            base = nt * CT
            pz = ppz.tile([2, CT], F32, tag="pz")
            for kc in range(KC):
                nc.tensor.matmul(pz[:, :], zrwS[:, 2 * kc:2 * kc + 2],
                                 inS[:, kc * TB + base: kc * TB + base + CT],
                                 start=(kc == 0), stop=False)
            # fold gate biases (w_b + u_b) in via a K=1 ones matmul
            nc.tensor.matmul(pz[:, :], gbW[:, :], onesS[0:1, 0:CT],
                             start=False, stop=True)
            nc.vector.tensor_copy(xzrS[:, base:base + CT], pz[:, :])
            for hc in range(KC):
                pq = ppq.tile([128, CT], F32, tag="pq")
                for ic in range(KC):
                    nc.tensor.matmul(pq[:, :], wxS[:, ic * H + hc * 128: ic * H + (hc + 1) * 128],
                                     inS[:, ic * TB + base: ic * TB + base + CT],
                                     start=(ic == 0), stop=(ic == KC - 1))
                # scatter into xnS[p, t*W + hc*BS + b] with per-partition bias h_w_b
                nrows = CT // BS
                t0 = base // BS
                xn_view = xnS[:, :].rearrange("p (t c b) -> p t c b", c=KC, b=BS)
                nc.vector.tensor_scalar_add(
                    xn_view[:, t0:t0 + nrows, hc, :],
                    pq[:, :].rearrange("p (t b) -> p t b", b=BS),
                    hwbS[:, hc:hc + 1])

    if dbg is not None:
        nc.sync.dma_start(dbg["xnS"], xnS[:, :])
        nc.sync.dma_start(dbg["xzrS"], xzrS[:, :])

    # ---------------- recurrence (single chain, latency-optimized) ---------
    # p_q is padded so each mc accumulation region sits in its own 2KB PSUM
    # bank (zero region) — all four groups stay open concurrently across the
    # e1/f split matmuls.
    QS = 512
    with tc.tile_pool(name="st", bufs=3) as st, \
         tc.tile_pool(name="pqp", bufs=1, space="PSUM") as pqp, \
         tc.tile_pool(name="pzp", bufs=1, space="PSUM") as pzp, \
         tc.tile_pool(name="pbp", bufs=1, space="PSUM") as pbp:
        e1_prev = f_prev = None
        for t in range(TS):
            prev = h0S[:, :] if t == 0 else outS[:, (t - 1) * W: t * W]

            # h(t) = e1_prev + f_prev, so every matmul against h(t) is split
            # into an e1 part (runs in the previous tanh's shadow) and an f
            # part (runs as soon as f lands) — the h' add is output-only.
            p_zr = pzp.tile([2, BS], F32, tag="p_zr")
            p_q = pqp.tile([128, KC * QS], F32, tag="p_q")
            if t == 0:
                for kc in range(KC):
                    nc.tensor.matmul(p_zr[:, :], zruS[:, 2 * kc:2 * kc + 2],
                                     prev[:, kc * BS:(kc + 1) * BS],
                                     start=(kc == 0), stop=False)
                nc.tensor.matmul(p_zr[:, :], i2S[:, :], xzrS[:, t * BS:(t + 1) * BS],
                                 start=False, stop=True)
                for mc in range(KC):
                    for kc in range(KC):
                        nc.tensor.matmul(p_q[:, mc * QS: mc * QS + BS],
                                         whS[:, kc * H + mc * 128: kc * H + (mc + 1) * 128],
                                         prev[:, kc * BS:(kc + 1) * BS],
                                         start=(kc == 0), stop=(kc == KC - 1))
            else:
                # x-injection first: no state dependency, fills PE idle time
                nc.tensor.matmul(p_zr[:, :], i2S[:, :], xzrS[:, t * BS:(t + 1) * BS],
                                 start=True, stop=False)
                # e1-part: main then gates (PE busy during tanh shadow)
                for mc in range(KC):
                    for kc in range(KC):
                        nc.tensor.matmul(p_q[:, mc * QS: mc * QS + BS],
                                         whS[:, kc * H + mc * 128: kc * H + (mc + 1) * 128],
                                         e1_prev[:, kc * BS:(kc + 1) * BS],
                                         start=(kc == 0), stop=False)
                for kc in range(KC):
                    nc.tensor.matmul(p_zr[:, :], zruS[:, 2 * kc:2 * kc + 2],
                                     e1_prev[:, kc * BS:(kc + 1) * BS],
                                     start=False, stop=False)
                # f-part: gates close the group, then main
                for kc in range(KC):
                    nc.tensor.matmul(p_zr[:, :], zruS[:, 2 * kc:2 * kc + 2],
                                     f_prev[:, kc * BS:(kc + 1) * BS],
                                     start=False, stop=(kc == KC - 1))
                for mc in range(KC):
                    for kc in range(KC):
                        nc.tensor.matmul(p_q[:, mc * QS: mc * QS + BS],
                                         whS[:, kc * H + mc * 128: kc * H + (mc + 1) * 128],
                                         f_prev[:, kc * BS:(kc + 1) * BS],
                                         start=False, stop=(kc == KC - 1))

            zr_s = st.tile([2, BS], BF16, tag="zr")
            nc.scalar.activation(zr_s[:, :], p_zr[:, :], AF.Sigmoid)
            zm_s = st.tile([1, BS], BF16, tag="zm")
            nc.scalar.activation(zm_s[:, :], p_zr[0:1, :], AF.Sigmoid, scale=-1.0)

            # broadcast z, r, (1-z) across partitions via PE selector matmuls.
            # Separate PSUM tiles so each consumer waits only its own matmul.
            rhs_zr = zr_s[:, :].unsqueeze(1).broadcast_to((2, KC, BS))
            rhs_zm = zm_s[:, :].unsqueeze(1).broadcast_to((1, KC, BS))
            rbP = pbp.tile([128, W], F32, tag="rbP")
            nc.tensor.matmul(rbP[:, :].rearrange("p (c b) -> p c b", b=BS),
                             selS[:, 128:256], rhs_zr, start=True, stop=True)
            zbP = pbp.tile([128, W], F32, tag="zbP")
            nc.tensor.matmul(zbP[:, :].rearrange("p (c b) -> p c b", b=BS),
                             selS[:, 0:128], rhs_zr, start=True, stop=True)
            zmP = pbp.tile([128, W], F32, tag="zmP")
            nc.tensor.matmul(zmP[:, :].rearrange("p (c b) -> p c b", b=BS),
                             onesS[0:1, 0:128], rhs_zm, start=True, stop=True)

            u = st.tile([128, W], F32, tag="u")
            nc.vector.tensor_add(
                u[:, :].rearrange("p (c b) -> p c b", b=BS),
                p_q[:, :].rearrange("p (c q) -> p c q", q=QS)[:, :, 0:BS],
                hubS[:, :].unsqueeze(2).broadcast_to((128, KC, BS)))
            s = st.tile([128, W], F32, tag="s")
            nc.vector.tensor_mul(s[:, :], u[:, :], rbP[:, :])
            q = st.tile([128, W], F32, tag="q")
            nc.vector.tensor_add(q[:, :], s[:, :], xnS[:, t * W:(t + 1) * W])
            # z*prev runs in the tanh shadow on DVE
            e1 = st.tile([128, W], BF16, tag="e1")
            nc.vector.tensor_mul(e1[:, :], prev, zbP[:, :])
            nt_ = st.tile([128, W], BF16, tag="nt")
            nc.scalar.activation(nt_[:, :], q[:, :], AF.Tanh)
            f_ = st.tile([128, W], BF16, tag="f")
            nc.vector.tensor_mul(f_[:, :], nt_[:, :], zmP[:, :])
            nc.vector.tensor_add(outS[:, t * W:(t + 1) * W], e1[:, :], f_[:, :])
            e1_prev, f_prev = e1, f_

            oc = min(64, TS)
            if (t + 1) % oc == 0:
                nc.sync.dma_start(d["outT"][:, (t + 1 - oc) * W:(t + 1) * W],
                                  outS[:, (t + 1 - oc) * W:(t + 1) * W])


def build_nc(TS: int = T, debug_dump: bool = False):
    nc = bacc.Bacc("TRN2", target_bir_lowering=False, debug=False)
    TB = TS * BS
    d = {}
    d["inT"] = nc.dram_tensor("inT", [I, TB], BF16, kind="ExternalInput").ap()
    d["h0T"] = nc.dram_tensor("h0T", [128, W], BF16, kind="ExternalInput").ap()
    d["whT"] = nc.dram_tensor("whT", [H, H], BF16, kind="ExternalInput").ap()
    d["wxT"] = nc.dram_tensor("wxT", [I, H], BF16, kind="ExternalInput").ap()
    d["zruT"] = nc.dram_tensor("zruT", [H, 2], BF16, kind="ExternalInput").ap()
    d["zrwT"] = nc.dram_tensor("zrwT", [I, 2], BF16, kind="ExternalInput").ap()
    d["hubT"] = nc.dram_tensor("hubT", [128, KC], F32, kind="ExternalInput").ap()
    d["hwbT"] = nc.dram_tensor("hwbT", [128, KC], F32, kind="ExternalInput").ap()
    d["gb"] = nc.dram_tensor("gb", [1, 2], BF16, kind="ExternalInput").ap()
    d["i2"] = nc.dram_tensor("i2", [2, 2], BF16, kind="ExternalInput").ap()
    d["sel2"] = nc.dram_tensor("sel2", [2, 256], BF16, kind="ExternalInput").ap()
    d["outT"] = nc.dram_tensor("outT", [128, TS * W], BF16, kind="ExternalOutput").ap()

    dbg = None
    if debug_dump:
        dbg = {
            "xnS": nc.dram_tensor("dbg_xnS", [128, TS * W], BF16, kind="ExternalOutput").ap(),
            "xzrS": nc.dram_tensor("dbg_xzrS", [1, TS * 2 * BS], BF16, kind="ExternalOutput").ap(),
            "p_q": nc.dram_tensor("dbg_p_q", [128, W], F32, kind="ExternalOutput").ap(),
            "p_b": nc.dram_tensor("dbg_p_b", [128, 2 * W], F32, kind="ExternalOutput").ap(),
            "zr_s": nc.dram_tensor("dbg_zr_s", [1, 2 * BS], BF16, kind="ExternalOutput").ap(),
            "q": nc.dram_tensor("dbg_q", [128, W], F32, kind="ExternalOutput").ap(),
            "nt": nc.dram_tensor("dbg_nt", [128, W], BF16, kind="ExternalOutput").ap(),
        }

    with tile.TileContext(nc) as tc, ExitStack() as ctx:
        _emit(ctx, tc, d, TS, dbg)
    nc.compile()
    return nc


def pack_inputs(inputs: dict, TS: int = T) -> list[dict]:
    """Host-side shard + relayout. Returns per-core in_maps."""
    f32 = np.float32
    inp = np.asarray(inputs["input"], f32)
    hid = np.asarray(inputs["hidden"], f32)
    wh = np.ascontiguousarray(np.asarray(inputs["h_u_w"], f32).T).astype(bf16)
    wx = np.ascontiguousarray(np.asarray(inputs["h_w_w"], f32).T).astype(bf16)
    zru = np.stack([np.asarray(inputs["zt_u_w"], f32)[0],
                    np.asarray(inputs["rt_u_w"], f32)[0]], axis=1).astype(bf16)
    zrw = np.stack([np.asarray(inputs["zt_w_w"], f32)[0],
                    np.asarray(inputs["rt_w_w"], f32)[0]], axis=1).astype(bf16)
    hub = np.ascontiguousarray(np.asarray(inputs["h_u_b"], f32).reshape(KC, 128).T)
    hwb = np.ascontiguousarray(np.asarray(inputs["h_w_b"], f32).reshape(KC, 128).T)
    gb = np.array([[float(inputs["zt_w_b"][0]) + float(inputs["zt_u_b"][0]),
                    float(inputs["rt_w_b"][0]) + float(inputs["rt_u_b"][0])]]).astype(bf16)
    i2 = np.eye(2, dtype=bf16)
    sel2 = np.zeros((2, 256), bf16)
    sel2[0, 0:128] = 1
    sel2[1, 128:256] = 1

    in_maps = []
    for c in range(NCORES):
        sl = inp[:TS, c * BS:(c + 1) * BS, :]                     # [TS, 8, I]
        inT = np.ascontiguousarray(sl.transpose(2, 0, 1).reshape(I, TS * BS)).astype(bf16)
        h0 = hid[c * BS:(c + 1) * BS, :]                          # [8, H]
        h0T = np.ascontiguousarray(
            h0.T.reshape(KC, 128, BS).transpose(1, 0, 2).reshape(128, W)).astype(bf16)
        in_maps.append({
            "inT": inT, "h0T": h0T, "whT": wh, "wxT": wx, "zruT": zru,
            "zrwT": zrw, "hubT": hub, "hwbT": hwb, "gb": gb, "i2": i2,
            "sel2": sel2,
        })
    return in_maps


def unpack_outputs(results: list[dict], TS: int = T):
    output = np.empty((TS, B, H), np.float32)
    for c in range(NCORES):
        o = np.asarray(results[c]["outT"]).reshape(128, TS, KC, BS)
        output[:, c * BS:(c + 1) * BS, :] = \
            o.transpose(1, 3, 2, 0).reshape(TS, BS, H).astype(np.float32)
    hidden_final = output[-1].copy()
    return output, hidden_final


_NC_CACHE = {}


def kernel(**inputs):
    if T not in _NC_CACHE:
        _NC_CACHE[T] = build_nc(T)
    nc = _NC_CACHE[T]
    in_maps = pack_inputs(inputs, T)
    res = run_bass_kernel_spmd(nc, in_maps, list(range(NCORES)))
    return unpack_outputs(res.results, T)
